# revision 1
# baseline (speedup 1.0000x reference)
"""Bass/Tile kernel for nn_CTransformer (3x3 neighborhood attention), TRN2.

Per-core layout: channel-on-partition. Core handles 32 image rows of one batch
(+1 halo row each side), width padded to 66. Tokens NT = 34*66 = 2244.

Pipeline:
  stats (PE ones-matmul) -> rstd/murstd -> Xhat = X*rstd (DVE)
  qkv = WallT.T @ Xhat + rank1(-wsum x murstd + bias)   (PE fp32r, psum)
  evac qkv -> bf16 sbuf
  per core-strip (4 x 512 tokens):
    prods = q * k_shift (DVE bf16), scores = mask.T @ prods (PE, psum [72,512])
    e = exp(scores) (ACT), den = tm.T @ e (PE), rden = 1/den (DVE)
    rdenrep = r72.T @ rden (PE), attn = e * rdenrep (DVE)
    per t: erep = rp_t.T @ attn (PE) -> evac -> avp = erep * v_shift (DVE)
           o += I.T @ avp (PE accumulate, psum)
    outproj: out = OWT.T @ o + bias + I.T @ x_center (PE), DMA out.
"""
import numpy as np
from contextlib import ExitStack

try:
    import concourse.bass as bass
except ImportError:
    import sys
    sys.path.insert(0, '/opt/trn_rl_repo')
    import concourse.bass as bass
import concourse.bacc as bacc
import concourse.tile as tile
from concourse import mybir

F32 = mybir.dt.float32
F32R = mybir.dt.float32r
BF16 = mybir.dt.bfloat16

B, C, H, W = 4, 256, 64, 64
NHEAD, DH = 8, 32
EPS = 1e-5
NCORE = 8
RPC = 32                 # image rows per core
R2, W2 = RPC + 2, W + 2  # 34 x 66 padded grid
NT = R2 * W2             # 2244
SS = 374                 # qkv token strip (6 strips)
NS = NT // SS            # 6
CS = 512                 # core-token strip (8 image rows)
NCS = 4                  # 4 core strips
NDB = 6                  # qkv dblocks of 128
OFFS = [(i, j) for i in range(3) for j in range(3)]

# repack-16 permutation: qkv row (chunk*128 + h*16 + d) <- channel (h*32 + chunk*16 + d)
PERM = np.zeros(C, dtype=np.int64)
for _k in range(2):
    for _h in range(8):
        for _d in range(16):
            PERM[_k * 128 + _h * 16 + _d] = _h * 32 + _k * 16 + _d


def host_prep(inputs):
    """Fold LN/scale/permutation into weights; build constant matrices."""
    f = np.float32
    ln_w = np.asarray(inputs['ln_w'], np.float64)
    ln_b = np.asarray(inputs['ln_b'], np.float64)
    ipw = np.asarray(inputs['in_proj_w'], np.float64)
    ipb = np.asarray(inputs['in_proj_b'], np.float64)
    opw = np.asarray(inputs['out_proj_w'], np.float64)
    opb = np.asarray(inputs['out_proj_b'], np.float64)
    scale = DH ** -0.5
    Wt = ipw * ln_w[None, :]
    bt = ipb + ipw @ ln_b
    Wt[:C] *= scale
    bt[:C] *= scale
    Wall = np.concatenate([Wt[i * C:(i + 1) * C][PERM] for i in range(3)], 0)  # [768,256]
    ball = np.concatenate([bt[i * C:(i + 1) * C][PERM] for i in range(3)], 0)
    wsum = Wall.sum(1)
    OWp = opw[:, PERM]

    import concourse.mybir as _mb
    bf = _mb.dt.np(_mb.dt.bfloat16)
    consts = {}
    consts['wt'] = np.ascontiguousarray(Wall.T).astype(bf)         # [256, 768] lhsT
    consts['rk1a'] = (-wsum).astype(bf)[None, :]                   # [1, 768]
    consts['rk1b'] = ball.astype(bf)[None, :]                      # [1, 768]
    consts['owt'] = np.ascontiguousarray(OWp.T).astype(bf)         # [256, 256] lhsT
    consts['ob'] = opb.astype(bf)[None, :]                         # [1, 256]
    consts['ones1'] = np.ones((128, 1), bf)
    consts['onesr'] = np.ones((1, 512), bf)
    # scores masks: for offset t, [128, 72] with row r -> col t*8 + r//16
    mk = np.zeros((128, 9 * 72), f)
    for t in range(9):
        mk[np.arange(128), t * 72 + t * 8 + np.arange(128) // 16] = 1
    consts['mk'] = mk.astype(bf)
    # stats strip-selector: for strip s, [128, 6] ones in col s
    sel6 = np.zeros((128, 6 * 6), f)
    for ss_ in range(6):
        sel6[:, ss_ * 6 + ss_] = 1
    consts['sel6'] = sel6
    # den sum: [72, 8]
    tm = np.zeros((72, 8), f)
    for t in range(9):
        for h in range(8):
            tm[t * 8 + h, h] = 1
    consts['tm'] = tm.astype(bf)
    # attn replication: [72, 9*128]
    rp = np.zeros((72, 9 * 128), f)
    for t in range(9):
        for c in range(128):
            rp[t * 8 + c // 16, t * 128 + c] = 1
    consts['rp'] = rp.astype(bf)
    # rden -> 72 rows: [8, 72]
    r72 = np.zeros((8, 72), f)
    for t in range(9):
        for h in range(8):
            r72[h, t * 8 + h] = 1
    consts['r72'] = r72.astype(bf)
    consts['ident'] = np.eye(128, dtype=f)
    return consts


def core_inputs(inputs, consts):
    """Build per-core in_maps (pad + slice on host)."""
    src = np.asarray(inputs['src'], np.float32)
    maps = []
    for core in range(NCORE):
        b = core // 2
        r0 = (core % 2) * RPC
        xp = np.zeros((C, R2, W2), np.float32)
        rlo, rhi = r0 - 1, r0 + RPC + 1
        slo, shi = max(rlo, 0), min(rhi, H)
        xp[:, (slo - rlo):(shi - rlo), 1:W + 1] = src[b, :, slo:shi, :]
        m = {'x': xp.reshape(C, NT)}
        m.update(consts)
        maps.append(m)
    return maps


BF = {
    F32: np.float32, BF16: np.float32,
}


def build(att_f32=False, dbg=None):
    """Build the Bass program. Returns nc. dbg in {None,'qkv','attn'} adds debug outputs."""
    AD = F32 if att_f32 else BF16    # attention dtype
    nc = bacc.Bacc('TRN2', target_bir_lowering=False, debug=False, num_devices=NCORE)

    x_d = nc.dram_tensor('x', [C, NT], F32, kind='ExternalInput')
    wt_d = nc.dram_tensor('wt', [C, 3 * C], BF16, kind='ExternalInput')
    rk1a_d = nc.dram_tensor('rk1a', [1, 3 * C], BF16, kind='ExternalInput')
    rk1b_d = nc.dram_tensor('rk1b', [1, 3 * C], BF16, kind='ExternalInput')
    sel6_d = nc.dram_tensor('sel6', [128, 36], F32, kind='ExternalInput')
    owt_d = nc.dram_tensor('owt', [C, C], BF16, kind='ExternalInput')
    ob_d = nc.dram_tensor('ob', [1, C], BF16, kind='ExternalInput')
    ones1_d = nc.dram_tensor('ones1', [128, 1], BF16, kind='ExternalInput')
    onesr_d = nc.dram_tensor('onesr', [1, 512], BF16, kind='ExternalInput')
    mk_d = nc.dram_tensor('mk', [128, 9 * 72], BF16, kind='ExternalInput')
    tm_d = nc.dram_tensor('tm', [72, 8], BF16, kind='ExternalInput')
    rp_d = nc.dram_tensor('rp', [72, 9 * 128], BF16, kind='ExternalInput')
    r72_d = nc.dram_tensor('r72', [8, 72], BF16, kind='ExternalInput')
    id_d = nc.dram_tensor('ident', [128, 128], F32, kind='ExternalInput')

    out_d = nc.dram_tensor('out', [C, RPC * W], F32, kind='ExternalOutput')
    if dbg == 'qkv':
        dqkv = nc.dram_tensor('dbg_qkv', [3 * C, NT], F32, kind='ExternalOutput')
    if dbg == 'attn':
        dattn = nc.dram_tensor('dbg_attn', [72, CS * NCS], F32, kind='ExternalOutput')

    r32 = lambda ap: ap.bitcast(F32R)

    with tile.TileContext(nc) as tc, ExitStack() as ctx:
        ctx.enter_context(nc.allow_low_precision(reason='bf16 attention path'))
        P = ctx.enter_context(tc.tile_pool(name='persist', bufs=1))
        T3 = ctx.enter_context(tc.tile_pool(name='work', bufs=3))
        PS = ctx.enter_context(tc.tile_pool(name='psum', bufs=1, space='PSUM'))

        mm = lambda *a, **kw: nc.tensor.matmul(*a, **kw)

        # ---- load constants + input ----
        xsb = [P.tile([128, NT], F32, tag=f'xsb{c}', name=f'xsb{c}') for c in range(2)]
        for c in range(2):
            nc.sync.dma_start(xsb[c][:], x_d[128 * c:128 * (c + 1), :])
        wt_c = [P.tile([128, 3 * C], BF16, tag=f'wt{c}', name=f'wt{c}') for c in range(2)]
        for c in range(2):
            nc.sync.dma_start(wt_c[c][:], wt_d[128 * c:128 * (c + 1), :])
        rk1a_s = P.tile([1, 3 * C], BF16, tag='rk1a')
        nc.sync.dma_start(rk1a_s[:], rk1a_d[:])
        rk1b_s = P.tile([1, 3 * C], BF16, tag='rk1b')
        nc.sync.dma_start(rk1b_s[:], rk1b_d[:])
        sel6_s = P.tile([128, 36], F32, tag='sel6')
        nc.sync.dma_start(sel6_s[:], sel6_d[:])
        owt_c = [P.tile([128, C], BF16, tag=f'owt{c}', name=f'owt{c}') for c in range(2)]
        for c in range(2):
            nc.sync.dma_start(owt_c[c][:], owt_d[128 * c:128 * (c + 1), :])
        ob_s = P.tile([1, C], BF16, tag='ob')
        nc.sync.dma_start(ob_s[:], ob_d[:])
        ones1_s = P.tile([128, 1], BF16, tag='ones1')
        nc.sync.dma_start(ones1_s[:], ones1_d[:])
        mk_s = P.tile([128, 9 * 72], AD, tag='mk')
        nc.sync.dma_start(mk_s[:], mk_d[:])
        tm_s = P.tile([72, 8], AD, tag='tm')
        nc.sync.dma_start(tm_s[:], tm_d[:])
        rp_s = P.tile([72, 9 * 128], AD, tag='rp')
        nc.sync.dma_start(rp_s[:], rp_d[:])
        r72_s = P.tile([8, 72], AD, tag='r72')
        nc.sync.dma_start(r72_s[:], r72_d[:])
        id_s = P.tile([128, 128], AD, tag='ident')
        nc.gpsimd.dma_start(id_s[:], id_d[:])
        idf_s = P.tile([128, 128], F32, tag='identf')
        nc.sync.dma_start(idf_s[:], id_d[:])

        onesr_s = P.tile([1, CS], BF16, tag='onesr')
        nc.sync.dma_start(onesr_s[:], onesr_d[:])
        ones374 = onesr_s[:, 0:SS]
        ones512 = onesr_s[:, 0:CS]

        # ---- stats + xhat ----
        xhat = [P.tile([128, NT], BF16, tag=f'xhat{c}', name=f'xhat{c}') for c in range(2)]
        murstd = P.tile([NS, SS], BF16, tag='murstd')
        rstd = P.tile([NS, SS], F32, tag='rstd')
        xb16 = [P.tile([128, NT], BF16, tag=f'xb16{c}', name=f'xb16{c}') for c in range(2)]
        sx6 = P.tile([NS, SS], F32, tag='sx6')
        sxx6 = P.tile([NS, SS], F32, tag='sxx6')
        for s in range(NS):
            sl = bass.ts(s, SS)
            xsq = [T3.tile([128, SS], BF16, tag='xsq', name='xsq') for _ in range(2)]
            for c in range(2):
                nc.scalar.activation(xb16[c][:, sl], xsb[c][:, sl],
                                     mybir.ActivationFunctionType.Copy)
                nc.scalar.square(xsq[c][:], xsb[c][:, sl])
            ps_sx = PS.tile([1, SS], F32, tag='psB', bufs=2, name='ps_sx')
            ps_sxx = PS.tile([1, SS], F32, tag='psB', bufs=2, name='ps_sxx')
            for c in range(2):
                mm(ps_sx[:], ones1_s[:], xb16[c][:, sl],
                   start=(c == 0), stop=(c == 1))
                mm(ps_sxx[:], ones1_s[:], xsq[c][:],
                   start=(c == 0), stop=(c == 1))
            tx = T3.tile([1, SS], F32, tag='txe', name='tx')
            txx = T3.tile([1, SS], F32, tag='txe', name='txx')
            nc.scalar.activation(tx[:], ps_sx[:], mybir.ActivationFunctionType.Copy)
            nc.scalar.activation(txx[:], ps_sxx[:], mybir.ActivationFunctionType.Copy)
            nc.sync.dma_start(sx6[s:s + 1, :], tx[:])
            nc.sync.dma_start(sxx6[s:s + 1, :], txx[:])
        # mu = Sx/C ; var = Sxx/C - mu^2 ; rstd = rsqrt(var+eps) ; murstd = mu*rstd
        mu = P.tile([NS, SS], F32, tag='mu')
        nc.vector.tensor_scalar_mul(mu[:], sx6[:], 1.0 / C)
        musq = T3.tile([NS, SS], F32, tag='musq')
        nc.vector.tensor_mul(musq[:], mu[:], mu[:])
        var = T3.tile([NS, SS], F32, tag='var')
        nc.vector.scalar_tensor_tensor(var[:], sxx6[:], 1.0 / C, musq[:],
                                       mybir.AluOpType.mult, mybir.AluOpType.subtract)
        epsc = P.tile([NS, 1], F32, tag='epsc')
        nc.gpsimd.memset(epsc[:], EPS)
        sd = T3.tile([NS, SS], F32, tag='sd')
        nc.scalar.activation(sd[:], var[:], mybir.ActivationFunctionType.Sqrt, bias=epsc[:])
        nc.vector.reciprocal(rstd[:], sd[:])
        nc.vector.tensor_mul(murstd[:], mu[:], rstd[:])
        murstd1 = P.tile([1, NT], BF16, tag='murstd1')
        nc.sync.dma_start(murstd1[:], murstd[:])
        rstd1 = P.tile([1, NT], F32, tag='rstd1')
        nc.sync.dma_start(rstd1[:], rstd[:])

        rstdb = [T3.tile([128, SS], F32, tag='rstdb', name='rstdb') for _ in range(NS)]
        for s in range(NS):
            nc.gpsimd.partition_broadcast(rstdb[s][:], rstd1[:, bass.ts(s, SS)])
            for c in range(2):
                nc.vector.tensor_mul(xhat[c][:, bass.ts(s, SS)],
                                     xsb[c][:, bass.ts(s, SS)], rstdb[s][:])

        # ---- qkv matmul + evac to bf16 ----
        qkv = [P.tile([128, NT], AD, tag=f'qkv{d}', name=f'qkv{d}') for d in range(NDB)]
        evac_rr = [0]

        def evac(dst_ap, src_ap):
            # round-robin ACT / DVE for psum->sbuf copies
            if evac_rr[0] % 2 == 0:
                nc.scalar.activation(dst_ap, src_ap, mybir.ActivationFunctionType.Copy)
            else:
                nc.vector.tensor_copy(dst_ap, src_ap)
            evac_rr[0] += 1

        for d in range(NDB):
            for s in range(NS):
                pq = PS.tile([128, SS], F32, tag='psA', bufs=2, name='pq')
                for c in range(2):
                    mm(pq[:], wt_c[c][:, bass.ts(d, 128)],
                       xhat[c][:, bass.ts(s, SS)], start=(c == 0), stop=False)
                mm(pq[:], rk1a_s[:, bass.ts(d, 128)], murstd1[:, bass.ts(s, SS)],
                   start=False, stop=False)
                mm(pq[:], rk1b_s[:, bass.ts(d, 128)], ones374,
                   start=False, stop=True)
                evac(qkv[d][:, bass.ts(s, SS)], pq[:])

        if dbg == 'qkv':
            for d in range(NDB):
                qf = T3.tile([128, NT], F32, tag='dbgq')
                nc.vector.tensor_copy(qf[:], qkv[d][:])
                nc.sync.dma_start(dqkv[128 * d:128 * (d + 1), :], qf[:])

        # views [128, 34, 66]
        g = lambda tn: tn.rearrange('p (r w) -> p r w', w=W2)
        xg = [g(xsb[c]) for c in range(2)]
        qg = [g(qkv[0]), g(qkv[1])]
        kg = [g(qkv[2]), g(qkv[3])]
        vg = [g(qkv[4]), g(qkv[5])]

        # ---- attention per core strip ----
        for cs in range(NCS):
            r0 = 1 + 8 * cs
            ps_sc = PS.tile([72, CS], F32, tag='psB', bufs=2, name='ps_sc')
            for t, (i, j) in enumerate(OFFS):
                prods = []
                for c in range(2):
                    prod = T3.tile([128, 8, W], AD, tag='prod', bufs=18, name='prod')
                    nc.vector.tensor_mul(
                        prod[:],
                        qg[c][:, r0:r0 + 8, 1:1 + W],
                        kg[c][:, r0 + i - 1:r0 + i + 7, j:j + W])
                    prods.append(prod)
                for c in range(2):
                    mm(ps_sc[:, :], mk_s[:, bass.ts(t, 72)], prods[c][:],
                       start=(t == 0 and c == 0), stop=(t == 8 and c == 1))
            e_sb = T3.tile([72, CS], AD, tag='e_sb')
            nc.scalar.activation(e_sb[:], ps_sc[:], mybir.ActivationFunctionType.Exp)
            ps_den = PS.tile([8, CS], F32, tag='psA', bufs=2, name='ps_den')
            mm(ps_den[:], tm_s[:], e_sb[:])
            rden = T3.tile([8, CS], AD, tag='rden')
            nc.vector.reciprocal(rden[:], ps_den[:])
            ps_r72 = PS.tile([72, CS], F32, tag='psA', bufs=2, name='ps_r72')
            mm(ps_r72[:], r72_s[:], rden[:])
            attn = T3.tile([72, CS], AD, tag='attn')
            nc.vector.tensor_mul(attn[:], e_sb[:], ps_r72[:])
            if dbg == 'attn':
                af = T3.tile([72, CS], F32, tag='dbga')
                nc.vector.tensor_copy(af[:], attn[:])
                nc.sync.dma_start(dattn[:, bass.ts(cs, CS)], af[:])

            o_ps = [PS.tile([128, CS], F32, tag=f'psO{c}', bufs=1, name=f'o_ps{c}') for c in range(2)]
            for t, (i, j) in enumerate(OFFS):
                ps_er = PS.tile([128, CS], F32, tag='psC', bufs=2, name='ps_er')
                mm(ps_er[:], rp_s[:, bass.ts(t, 128)], attn[:])
                erep = T3.tile([128, CS], AD, tag='erep', bufs=6)
                evac(erep[:], ps_er[:])
                avps = []
                for c in range(2):
                    avp = T3.tile([128, 8, W], AD, tag='avp', bufs=12, name='avp')
                    nc.vector.tensor_mul(
                        avp[:], erep[:].rearrange('p (r w) -> p r w', w=W),
                        vg[c][:, r0 + i - 1:r0 + i + 7, j:j + W])
                    avps.append(avp)
                for c in range(2):
                    mm(o_ps[c][:], id_s[:], avps[c][:],
                       start=(t == 0), stop=(t == 8))
            o_sb = [T3.tile([128, CS], BF16, tag=f'o_sb{c}', name=f'o_sb{c}') for c in range(2)]
            for c in range(2):
                evac(o_sb[c][:], o_ps[c][:])

            # ---- out projection + bias + residual ----
            for db in range(2):
                op_ps = PS.tile([128, CS], F32, tag='psA', bufs=2, name='op_ps')
                for c in range(2):
                    mm(op_ps[:], owt_c[c][:, bass.ts(db, 128)],
                       o_sb[c][:], start=(c == 0), stop=False)
                mm(op_ps[:], ob_s[0:1, bass.ts(db, 128)], ones512,
                   start=False, stop=False)
                mm(op_ps[:], idf_s[:],
                   xg[db][:, r0:r0 + 8, 1:1 + W],
                   start=False, stop=True)
                ot = T3.tile([128, CS], F32, tag='ot', bufs=2)
                evac(ot[:], op_ps[:])
                nc.sync.dma_start(out_d[bass.ts(db, 128), bass.ts(cs, CS)], ot[:])

    nc.compile()
    return nc


_NC_CACHE = {}


def _get_nc(att_f32=False):
    key = ('nc', att_f32)
    if key not in _NC_CACHE:
        _NC_CACHE[key] = build(att_f32=att_f32)
    return _NC_CACHE[key]


def kernel(**inputs):
    """Full-input, full-output entry point. Shards over 8 NeuronCores."""
    from concourse.bass_utils import run_bass_kernel_spmd
    nc = _get_nc()
    consts = host_prep(inputs)
    maps = core_inputs(inputs, consts)
    res = run_bass_kernel_spmd(nc, maps, core_ids=list(range(NCORE)))
    out = np.zeros((B, C, H, W), np.float32)
    for core in range(NCORE):
        b = core // 2
        r0 = (core % 2) * RPC
        out[b, :, r0:r0 + RPC, :] = res.results[core]['out'].reshape(C, RPC, W)
    return out



# revision 4
# speedup vs baseline: 1.1652x; 1.1652x over previous
"""Bass/Tile kernel for nn_CTransformer (3x3 neighborhood attention), TRN2.

Per-core layout: channel-on-partition. Core handles 32 image rows of one batch
(+1 halo row each side), width padded to 66. Tokens NT = 34*66 = 2244.

v2 design notes:
  - PE warmup matmuls at t=0 (overlap input DMA) so HAM un-throttles to
    2.4 GHz before the qkv phase.
  - qkv = 3 matmul streams per (dblock, strip): 2 weight halves + one rank-2
    stream carrying both the mean correction (-wsum x murstd) and the bias.
  - LN rstd via Sqrt + reciprocal_approx_fast (no slow DVE reciprocal);
    softmax denominator likewise.
  - rstd broadcast to 128 partitions via a K=1 ones matmul (PE) instead of
    gpsimd partition_broadcast.
  - q/k/v stored as [128, 2, NT] (channel-half axis inside the tile) so each
    q*k product is ONE tensor_tensor op over both halves (FD=1024, 2x mode).
  - out-proj bias + residual folded into one scalar_tensor_tensor evac.
"""
import numpy as np
from contextlib import ExitStack

try:
    import concourse.bass as bass
except ImportError:
    import sys
    sys.path.insert(0, '/opt/trn_rl_repo')
    import concourse.bass as bass
import concourse.bacc as bacc
import concourse.tile as tile
from concourse import mybir

F32 = mybir.dt.float32
BF16 = mybir.dt.bfloat16

B, C, H, W = 4, 256, 64, 64
NHEAD, DH = 8, 32
EPS = 1e-5
NCORE = 8
RPC = 32                 # image rows per core
R2, W2 = RPC + 2, W + 2  # 34 x 66 padded grid
NT = R2 * W2             # 2244
SS = 374                 # qkv token strip (6 strips)
NS = NT // SS            # 6
CS = 512                 # core-token strip (8 image rows)
NCS = 4                  # 4 core strips
OFFS = [(i, j) for i in range(3) for j in range(3)]
AD = BF16

# repack-16 permutation: qkv row (chunk*128 + h*16 + d) <- channel (h*32 + chunk*16 + d)
PERM = np.zeros(C, dtype=np.int64)
for _k in range(2):
    for _h in range(8):
        for _d in range(16):
            PERM[_k * 128 + _h * 16 + _d] = _h * 32 + _k * 16 + _d


def host_prep(inputs):
    """Fold LN/scale/permutation into weights; build constant matrices."""
    f = np.float32
    ln_w = np.asarray(inputs['ln_w'], np.float64)
    ln_b = np.asarray(inputs['ln_b'], np.float64)
    ipw = np.asarray(inputs['in_proj_w'], np.float64)
    ipb = np.asarray(inputs['in_proj_b'], np.float64)
    opw = np.asarray(inputs['out_proj_w'], np.float64)
    opb = np.asarray(inputs['out_proj_b'], np.float64)
    scale = DH ** -0.5
    Wt = ipw * ln_w[None, :]
    bt = ipb + ipw @ ln_b
    Wt[:C] *= scale
    bt[:C] *= scale
    Wall = np.concatenate([Wt[i * C:(i + 1) * C][PERM] for i in range(3)], 0)  # [768,256]
    ball = np.concatenate([bt[i * C:(i + 1) * C][PERM] for i in range(3)], 0)
    wsum = Wall.sum(1)
    OWp = opw[:, PERM]

    import concourse.mybir as _mb
    bf = _mb.dt.np(_mb.dt.bfloat16)
    consts = {}
    consts['wt'] = np.ascontiguousarray(Wall.T).astype(bf)         # [256, 768] lhsT
    consts['rk2'] = np.stack([-wsum, ball]).astype(bf)             # [2, 768] lhsT
    consts['owt'] = np.ascontiguousarray(OWp.T).astype(bf)         # [256, 256] lhsT
    consts['obc'] = opb.astype(f).reshape(2, 128, 1)               # [2][128,1] bias cols
    consts['ones1'] = np.ones((128, 1), bf)
    consts['onesr'] = np.ones((1, 128), bf)
    # scores masks: for offset t, [128, 72] with row r -> col t*8 + r//16
    mk = np.zeros((128, 9 * 72), f)
    for t in range(9):
        mk[np.arange(128), t * 72 + t * 8 + np.arange(128) // 16] = 1
    consts['mk'] = mk.astype(bf)
    # den sum: [72, 8]
    tm = np.zeros((72, 8), f)
    for t in range(9):
        for h in range(8):
            tm[t * 8 + h, h] = 1
    consts['tm'] = tm.astype(bf)
    # attn replication: [72, 9*128]
    rp = np.zeros((72, 9 * 128), f)
    for t in range(9):
        for c in range(128):
            rp[t * 8 + c // 16, t * 128 + c] = 1
    consts['rp'] = rp.astype(bf)
    # rden -> 72 rows: [8, 72]
    r72 = np.zeros((8, 72), f)
    for t in range(9):
        for h in range(8):
            r72[h, t * 8 + h] = 1
    consts['r72'] = r72.astype(bf)
    consts['ident'] = np.eye(128, dtype=f).astype(bf)
    return consts


def core_inputs(inputs, consts):
    """Build per-core in_maps (pad + slice on host)."""
    src = np.asarray(inputs['src'], np.float32)
    maps = []
    for core in range(NCORE):
        b = core // 2
        r0 = (core % 2) * RPC
        xp = np.zeros((C, R2, W2), np.float32)
        rlo, rhi = r0 - 1, r0 + RPC + 1
        slo, shi = max(rlo, 0), min(rhi, H)
        xp[:, (slo - rlo):(shi - rlo), 1:W + 1] = src[b, :, slo:shi, :]
        m = {'x': xp.reshape(C, NT)}
        m.update(consts)
        maps.append(m)
    return maps


def build(dbg=None):
    nc = bacc.Bacc('TRN2', target_bir_lowering=False, debug=False, num_devices=NCORE)

    x_d = nc.dram_tensor('x', [C, NT], F32, kind='ExternalInput')
    wt_d = nc.dram_tensor('wt', [C, 3 * C], BF16, kind='ExternalInput')
    rk2_d = nc.dram_tensor('rk2', [2, 3 * C], BF16, kind='ExternalInput')
    owt_d = nc.dram_tensor('owt', [C, C], BF16, kind='ExternalInput')
    obc_d = nc.dram_tensor('obc', [2, 128, 1], F32, kind='ExternalInput')
    ones1_d = nc.dram_tensor('ones1', [128, 1], BF16, kind='ExternalInput')
    onesr_d = nc.dram_tensor('onesr', [1, 128], BF16, kind='ExternalInput')
    mk_d = nc.dram_tensor('mk', [128, 9 * 72], BF16, kind='ExternalInput')
    tm_d = nc.dram_tensor('tm', [72, 8], BF16, kind='ExternalInput')
    rp_d = nc.dram_tensor('rp', [72, 9 * 128], BF16, kind='ExternalInput')
    r72_d = nc.dram_tensor('r72', [8, 72], BF16, kind='ExternalInput')
    id_d = nc.dram_tensor('ident', [128, 128], BF16, kind='ExternalInput')

    out_d = nc.dram_tensor('out', [C, RPC * W], F32, kind='ExternalOutput')
    if dbg == 'qkv':
        dqkv = nc.dram_tensor('dbg_qkv', [3 * C, NT], F32, kind='ExternalOutput')

    with tile.TileContext(nc) as tc, ExitStack() as ctx:
        ctx.enter_context(nc.allow_low_precision(reason='bf16 attention path'))
        P = ctx.enter_context(tc.tile_pool(name='persist', bufs=1))
        T3 = ctx.enter_context(tc.tile_pool(name='work', bufs=3))
        PS = ctx.enter_context(tc.tile_pool(name='psum', bufs=1, space='PSUM'))

        mm = lambda *a, **kw: nc.tensor.matmul(*a, **kw)

        # ---- PE warmup: keep HAM busy while DMA streams inputs ----
        warm = P.tile([128, 512], BF16, tag='warm')
        nc.gpsimd.memset(warm[:], 0.0)
        for i in range(24):
            wps = PS.tile([128, 512], F32, tag='psO0', bufs=1, name='wps')
            mm(wps[:], warm[:, 0:128], warm[:], start=True, stop=True)

        # ---- constant loads ----
        wt_c = [P.tile([128, 3 * C], BF16, tag=f'wt{c}', name=f'wt{c}') for c in range(2)]
        for c in range(2):
            nc.sync.dma_start(wt_c[c][:], wt_d[128 * c:128 * (c + 1), :])
        rk2_s = P.tile([2, 3 * C], BF16, tag='rk2')
        nc.sync.dma_start(rk2_s[:], rk2_d[:])
        owt_c = [P.tile([128, C], BF16, tag=f'owt{c}', name=f'owt{c}') for c in range(2)]
        for c in range(2):
            nc.sync.dma_start(owt_c[c][:], owt_d[128 * c:128 * (c + 1), :])
        obc_s = [P.tile([128, 1], F32, tag=f'obc{c}', name=f'obc{c}') for c in range(2)]
        for c in range(2):
            nc.sync.dma_start(obc_s[c][:], obc_d[c])
        ones1_s = P.tile([128, 1], BF16, tag='ones1')
        nc.sync.dma_start(ones1_s[:], ones1_d[:])
        onesr_s = P.tile([1, 128], BF16, tag='onesr')
        nc.sync.dma_start(onesr_s[:], onesr_d[:])
        mk_s = P.tile([128, 9 * 72], AD, tag='mk')
        nc.sync.dma_start(mk_s[:], mk_d[:])
        tm_s = P.tile([72, 8], AD, tag='tm')
        nc.sync.dma_start(tm_s[:], tm_d[:])
        rp_s = P.tile([72, 9 * 128], AD, tag='rp')
        nc.sync.dma_start(rp_s[:], rp_d[:])
        r72_s = P.tile([8, 72], AD, tag='r72')
        nc.sync.dma_start(r72_s[:], r72_d[:])
        id_s = P.tile([128, 128], AD, tag='ident')
        nc.sync.dma_start(id_s[:], id_d[:])

        # ---- input x: per (half, strip) chunks so stats start early ----
        xsb = [P.tile([128, NT], F32, tag=f'xsb{c}', name=f'xsb{c}') for c in range(2)]
        for s in range(NS):
            for c in range(2):
                nc.sync.dma_start(xsb[c][:, bass.ts(s, SS)],
                                  x_d[128 * c:128 * (c + 1), bass.ts(s, SS)])

        evac_rr = [0]

        def evac(dst_ap, src_ap):
            # round-robin ACT / DVE for psum->sbuf copies
            if evac_rr[0] % 2 == 0:
                nc.scalar.activation(dst_ap, src_ap, mybir.ActivationFunctionType.Copy)
            else:
                nc.vector.tensor_copy(dst_ap, src_ap)
            evac_rr[0] += 1

        # ---- stats: Sx, Sxx per strip -> mu/var/rstd/murstd ----
        xb16 = [P.tile([128, NT], BF16, tag=f'xb16{c}', name=f'xb16{c}') for c in range(2)]
        mu6 = P.tile([NS, SS], F32, tag='mu6')
        m26 = P.tile([NS, SS], F32, tag='m26')
        for s in range(NS):
            sl = bass.ts(s, SS)
            xsq = [T3.tile([128, SS], BF16, tag='xsq', name='xsq', bufs=4) for _ in range(2)]
            for c in range(2):
                nc.vector.tensor_copy(xb16[c][:, sl], xsb[c][:, sl])
                nc.scalar.square(xsq[c][:], xsb[c][:, sl])
            ps_sx = PS.tile([1, SS], F32, tag='psB', bufs=2, name='ps_sx')
            ps_sxx = PS.tile([1, SS], F32, tag='psB', bufs=2, name='ps_sxx')
            for c in range(2):
                mm(ps_sx[:], ones1_s[:], xb16[c][:, sl], start=(c == 0), stop=(c == 1))
                mm(ps_sxx[:], ones1_s[:], xsq[c][:], start=(c == 0), stop=(c == 1))
            # evac scaled by 1/C, then DMA into row s (engines can't write at
            # partition offset s directly)
            tmu = T3.tile([1, SS], F32, tag='txe', name='tmu', bufs=4)
            tm2 = T3.tile([1, SS], F32, tag='txe', name='tm2', bufs=4)
            nc.scalar.activation(tmu[:], ps_sx[:],
                                 mybir.ActivationFunctionType.Copy, scale=1.0 / C)
            nc.scalar.activation(tm2[:], ps_sxx[:],
                                 mybir.ActivationFunctionType.Copy, scale=1.0 / C)
            nc.sync.dma_start(mu6[s:s + 1, :], tmu[:])
            nc.sync.dma_start(m26[s:s + 1, :], tm2[:])
        musq = T3.tile([NS, SS], F32, tag='musq')
        nc.vector.tensor_mul(musq[:], mu6[:], mu6[:])
        var = T3.tile([NS, SS], F32, tag='var')
        nc.vector.tensor_sub(var[:], m26[:], musq[:])
        epsc = P.tile([NS, 1], F32, tag='epsc')
        nc.gpsimd.memset(epsc[:], EPS)
        sd = T3.tile([NS, SS], F32, tag='sd')
        nc.scalar.activation(sd[:], var[:], mybir.ActivationFunctionType.Sqrt, bias=epsc[:])
        rstd = P.tile([NS, SS], F32, tag='rstd')
        nc.vector.reciprocal_approx_fast(rstd[:], sd[:])
        murstd = P.tile([NS, SS], BF16, tag='murstd')
        nc.vector.tensor_mul(murstd[:], mu6[:], rstd[:])
        rstdbf = P.tile([NS, SS], BF16, tag='rstdbf')
        nc.vector.tensor_copy(rstdbf[:], rstd[:])
        # rank-2 rhs: row0 = murstd, row1 = ones
        r2 = P.tile([2, NT], BF16, tag='r2')
        nc.gpsimd.memset(r2[:], 1.0)
        nc.sync.dma_start(r2[0:1, :], murstd[:])
        rstd1 = P.tile([1, NT], BF16, tag='rstd1')
        nc.sync.dma_start(rstd1[:], rstdbf[:])

        # ---- xhat = xb16 * rstd (broadcast via K=1 ones matmul) ----
        xhat = [P.tile([128, NT], BF16, tag=f'xhat{c}', name=f'xhat{c}') for c in range(2)]
        for s in range(NS):
            sl = bass.ts(s, SS)
            ps_rb = PS.tile([128, SS], F32, tag='psA', bufs=2, name='ps_rb')
            mm(ps_rb[:], onesr_s[:], rstd1[:, sl], start=True, stop=True)
            rstdb = T3.tile([128, SS], BF16, tag='rstdb', name='rstdb', bufs=3)
            nc.scalar.activation(rstdb[:], ps_rb[:], mybir.ActivationFunctionType.Copy)
            for c in range(2):
                nc.vector.tensor_mul(xhat[c][:, sl], xb16[c][:, sl], rstdb[:])

        # ---- qkv: psum alternates psA/psC tags for 4-deep pipelining ----
        qkv = [P.tile([128, 2, NT], AD, tag=f'qkv{t}', name=f'qkv{t}') for t in range(3)]
        for d in range(6):
            dl = bass.ts(d, 128)
            tens, half = qkv[d // 2], d % 2
            for s in range(NS):
                sl = bass.ts(s, SS)
                pq = PS.tile([128, SS], F32, tag=('psA' if s % 2 == 0 else 'psC'),
                             bufs=2, name='pq')
                for c in range(2):
                    mm(pq[:], wt_c[c][:, dl], xhat[c][:, sl], start=(c == 0), stop=False)
                mm(pq[:], rk2_s[:, dl], r2[:, sl], start=False, stop=True)
                evac(tens[:, half, sl], pq[:])

        if dbg == 'qkv':
            for d in range(6):
                qf = T3.tile([128, NT], F32, tag='dbgq')
                nc.vector.tensor_copy(qf[:], qkv[d // 2][:, d % 2, :])
                nc.sync.dma_start(dqkv[128 * d:128 * (d + 1), :], qf[:])

        # views [128, 2, 34, 66]
        g = lambda tn: tn[:].rearrange('p h (r w) -> p h r w', w=W2)
        qg, kg, vg = g(qkv[0]), g(qkv[1]), g(qkv[2])
        xg = [xsb[c][:].rearrange('p (r w) -> p r w', w=W2) for c in range(2)]

        # ---- attention per core strip ----
        for cs in range(NCS):
            r0 = 1 + 8 * cs
            ps_sc = PS.tile([72, CS], F32, tag='psB', bufs=2, name='ps_sc')
            for t, (i, j) in enumerate(OFFS):
                prod = T3.tile([128, 2, 8, W], AD, tag='prod', bufs=4, name='prod')
                nc.vector.tensor_mul(
                    prod[:],
                    qg[:, :, r0:r0 + 8, 1:1 + W],
                    kg[:, :, r0 + i - 1:r0 + i + 7, j:j + W])
                for c in range(2):
                    mm(ps_sc[:, :], mk_s[:, bass.ts(t, 72)], prod[:, c],
                       start=(t == 0 and c == 0), stop=(t == 8 and c == 1))
            e_sb = T3.tile([72, CS], AD, tag='e_sb')
            nc.scalar.activation(e_sb[:], ps_sc[:], mybir.ActivationFunctionType.Exp)
            ps_den = PS.tile([8, CS], F32, tag='psA', bufs=2, name='ps_den')
            mm(ps_den[:], tm_s[:], e_sb[:], start=True, stop=True)
            rdenf = T3.tile([8, CS], F32, tag='rdenf')
            nc.vector.reciprocal_approx_fast(rdenf[:], ps_den[:])
            rden = T3.tile([8, CS], AD, tag='rden')
            nc.vector.tensor_copy(rden[:], rdenf[:])
            ps_r72 = PS.tile([72, CS], F32, tag='psA', bufs=2, name='ps_r72')
            mm(ps_r72[:], r72_s[:], rden[:], start=True, stop=True)
            attn = T3.tile([72, CS], AD, tag='attn')
            nc.vector.tensor_mul(attn[:], e_sb[:], ps_r72[:])

            o_ps = [PS.tile([128, CS], F32, tag=f'psO{c}', bufs=1, name=f'o_ps{c}')
                    for c in range(2)]
            for t, (i, j) in enumerate(OFFS):
                ps_er = PS.tile([128, CS], F32, tag='psC', bufs=2, name='ps_er')
                mm(ps_er[:], rp_s[:, bass.ts(t, 128)], attn[:], start=True, stop=True)
                erep = T3.tile([128, CS], AD, tag='erep', bufs=4)
                evac(erep[:], ps_er[:])
                avp = T3.tile([128, 2, 8, W], AD, tag='avp', bufs=4, name='avp')
                erv = erep[:].rearrange('p (r w) -> p r w', w=W)
                for c in range(2):
                    nc.vector.tensor_mul(
                        avp[:, c], erv,
                        vg[:, c, r0 + i - 1:r0 + i + 7, j:j + W])
                for c in range(2):
                    mm(o_ps[c][:], id_s[:], avp[:, c], start=(t == 0), stop=(t == 8))
            o_sb = [T3.tile([128, CS], BF16, tag=f'o_sb{c}', name=f'o_sb{c}')
                    for c in range(2)]
            for c in range(2):
                evac(o_sb[c][:], o_ps[c][:])

            # ---- out projection; bias + residual folded into evac ----
            for db in range(2):
                op_ps = PS.tile([128, CS], F32, tag='psA', bufs=2, name='op_ps')
                for c in range(2):
                    mm(op_ps[:], owt_c[c][:, bass.ts(db, 128)], o_sb[c][:],
                       start=(c == 0), stop=(c == 1))
                ot = T3.tile([128, 8, W], F32, tag='ot', bufs=2)
                nc.vector.scalar_tensor_tensor(
                    ot[:], op_ps[:].rearrange('p (r w) -> p r w', w=W),
                    obc_s[db][:], xg[db][:, r0:r0 + 8, 1:1 + W],
                    mybir.AluOpType.add, mybir.AluOpType.add)
                nc.sync.dma_start(out_d[bass.ts(db, 128), bass.ts(cs, CS)], ot[:])

    nc.compile()
    return nc


_NC_CACHE = {}


def _get_nc(**kw):
    key = ('nc',) + tuple(sorted(kw.items()))
    if key not in _NC_CACHE:
        _NC_CACHE[key] = build(**kw)
    return _NC_CACHE[key]


def kernel(**inputs):
    """Full-input, full-output entry point. Shards over 8 NeuronCores."""
    from concourse.bass_utils import run_bass_kernel_spmd
    nc = _get_nc()
    consts = host_prep(inputs)
    maps = core_inputs(inputs, consts)
    res = run_bass_kernel_spmd(nc, maps, core_ids=list(range(NCORE)))
    out = np.zeros((B, C, H, W), np.float32)
    for core in range(NCORE):
        b = core // 2
        r0 = (core % 2) * RPC
        out[b, :, r0:r0 + RPC, :] = res.results[core]['out'].reshape(C, RPC, W)
    return out


# revision 8
# speedup vs baseline: 1.2042x; 1.0335x over previous
"""Bass/Tile kernel for nn_CTransformer (3x3 neighborhood attention), TRN2.

Per-core layout: channel-on-partition. Core handles 32 image rows of one batch
(+1 halo row each side), width padded to 66. Tokens NT = 34*66 = 2244.

v3 design notes:
  - strip-outer pipeline: stats -> LN -> xhat -> qkv per 374-token strip, so
    DMA, stats math and qkv matmuls overlap instead of running as serial
    phases.
  - x input DMA split across both HWDGE queues (sync + scalar).
  - qkv = 3 matmul streams per (dblock, strip): 2 weight halves + one rank-2
    stream carrying the mean correction (-wsum x murstd) and the bias.
  - LN rstd / softmax denominator via reciprocal_approx_fast.
  - erep (attention-weight replication 72 -> 128 rows) via broadcast DMA on
    the HWDGE queues instead of PE matmul + psum evac.
  - q/k/v stored as [128, 2, NT]; each q*k product is one tensor_tensor op
    over both halves (FD=1024, 2x mode).
  - out-proj bias + residual folded into one scalar_tensor_tensor evac.
"""
import numpy as np
from contextlib import ExitStack

try:
    import concourse.bass as bass
except ImportError:
    import sys
    sys.path.insert(0, '/opt/trn_rl_repo')
    import concourse.bass as bass
import concourse.bacc as bacc
import concourse.tile as tile
from concourse import mybir

F32 = mybir.dt.float32
BF16 = mybir.dt.bfloat16

B, C, H, W = 4, 256, 64, 64
NHEAD, DH = 8, 32
EPS = 1e-5
NCORE = 8
RPC = 32                 # image rows per core
R2, W2 = RPC + 2, W + 2  # 34 x 66 padded grid
NT = R2 * W2             # 2244
SS = 374                 # qkv token strip (6 strips)
NS = NT // SS            # 6
CS = 512                 # core-token strip (8 image rows)
NCS = 4                  # 4 core strips
OFFS = [(i, j) for i in range(3) for j in range(3)]
AD = BF16
EREP_DMA = False          # replicate attn rows via broadcast DMA (vs PE matmul)

# repack-16 permutation: qkv row (chunk*128 + h*16 + d) <- channel (h*32 + chunk*16 + d)
PERM = np.zeros(C, dtype=np.int64)
for _k in range(2):
    for _h in range(8):
        for _d in range(16):
            PERM[_k * 128 + _h * 16 + _d] = _h * 32 + _k * 16 + _d


def host_prep(inputs):
    """Fold LN/scale/permutation into weights; build constant matrices."""
    f = np.float32
    ln_w = np.asarray(inputs['ln_w'], np.float64)
    ln_b = np.asarray(inputs['ln_b'], np.float64)
    ipw = np.asarray(inputs['in_proj_w'], np.float64)
    ipb = np.asarray(inputs['in_proj_b'], np.float64)
    opw = np.asarray(inputs['out_proj_w'], np.float64)
    opb = np.asarray(inputs['out_proj_b'], np.float64)
    scale = DH ** -0.5
    Wt = ipw * ln_w[None, :]
    bt = ipb + ipw @ ln_b
    Wt[:C] *= scale
    bt[:C] *= scale
    Wall = np.concatenate([Wt[i * C:(i + 1) * C][PERM] for i in range(3)], 0)  # [768,256]
    ball = np.concatenate([bt[i * C:(i + 1) * C][PERM] for i in range(3)], 0)
    wsum = Wall.sum(1)
    OWp = opw[:, PERM]

    import concourse.mybir as _mb
    bf = _mb.dt.np(_mb.dt.bfloat16)
    consts = {}
    consts['wt'] = np.ascontiguousarray(Wall.T).astype(bf)         # [256, 768] lhsT
    consts['rk2'] = np.stack([-wsum, ball]).astype(bf)             # [2, 768] lhsT
    consts['owt'] = np.ascontiguousarray(OWp.T).astype(bf)         # [256, 256] lhsT
    consts['obc'] = opb.astype(f).reshape(2, 128, 1)               # [2][128,1] bias cols
    consts['ones1'] = np.ones((128, 1), bf)
    consts['onesr'] = np.ones((1, 128), bf)
    consts['onesnt'] = np.ones((1, NT), bf)
    consts['epsb'] = np.full((3, 1), EPS, f)
    # scores masks: for offset t, [128, 72] with row r -> col t*8 + r//16
    mk = np.zeros((128, 9 * 72), f)
    for t in range(9):
        mk[np.arange(128), t * 72 + t * 8 + np.arange(128) // 16] = 1
    consts['mk'] = mk.astype(bf)
    # den sum: [72, 8]
    tm = np.zeros((72, 8), f)
    for t in range(9):
        for h in range(8):
            tm[t * 8 + h, h] = 1
    consts['tm'] = tm.astype(bf)
    # attn replication: [72, 9*128]
    rp = np.zeros((72, 9 * 128), f)
    for t in range(9):
        for c in range(128):
            rp[t * 8 + c // 16, t * 128 + c] = 1
    consts['rp'] = rp.astype(bf)
    # rden -> 72 rows: [8, 72]
    r72 = np.zeros((8, 72), f)
    for t in range(9):
        for h in range(8):
            r72[h, t * 8 + h] = 1
    consts['r72'] = r72.astype(bf)
    consts['ident'] = np.eye(128, dtype=f).astype(bf)
    return consts


def core_inputs(inputs, consts):
    """Build per-core in_maps (pad + slice on host)."""
    src = np.asarray(inputs['src'], np.float32)
    maps = []
    for core in range(NCORE):
        b = core // 2
        r0 = (core % 2) * RPC
        xp = np.zeros((C, R2, W2), np.float32)
        rlo, rhi = r0 - 1, r0 + RPC + 1
        slo, shi = max(rlo, 0), min(rhi, H)
        xp[:, (slo - rlo):(shi - rlo), 1:W + 1] = src[b, :, slo:shi, :]
        m = {'x': xp.reshape(C, NT)}
        m.update(consts)
        maps.append(m)
    return maps


def build(erep_dma=EREP_DMA):
    nc = bacc.Bacc('TRN2', target_bir_lowering=False, debug=False, num_devices=NCORE)

    x_d = nc.dram_tensor('x', [C, NT], F32, kind='ExternalInput')
    wt_d = nc.dram_tensor('wt', [C, 3 * C], BF16, kind='ExternalInput')
    rk2_d = nc.dram_tensor('rk2', [2, 3 * C], BF16, kind='ExternalInput')
    owt_d = nc.dram_tensor('owt', [C, C], BF16, kind='ExternalInput')
    obc_d = nc.dram_tensor('obc', [2, 128, 1], F32, kind='ExternalInput')
    ones1_d = nc.dram_tensor('ones1', [128, 1], BF16, kind='ExternalInput')
    onesr_d = nc.dram_tensor('onesr', [1, 128], BF16, kind='ExternalInput')
    onesnt_d = nc.dram_tensor('onesnt', [1, NT], BF16, kind='ExternalInput')
    epsb_d = nc.dram_tensor('epsb', [3, 1], F32, kind='ExternalInput')
    mk_d = nc.dram_tensor('mk', [128, 9 * 72], BF16, kind='ExternalInput')
    tm_d = nc.dram_tensor('tm', [72, 8], BF16, kind='ExternalInput')
    rp_d = nc.dram_tensor('rp', [72, 9 * 128], BF16, kind='ExternalInput')
    r72_d = nc.dram_tensor('r72', [8, 72], BF16, kind='ExternalInput')
    id_d = nc.dram_tensor('ident', [128, 128], BF16, kind='ExternalInput')

    out_d = nc.dram_tensor('out', [C, RPC * W], F32, kind='ExternalOutput')

    with tile.TileContext(nc) as tc, ExitStack() as ctx:
        ctx.enter_context(nc.allow_low_precision(reason='bf16 attention path'))
        P = ctx.enter_context(tc.tile_pool(name='persist', bufs=1))
        T3 = ctx.enter_context(tc.tile_pool(name='work', bufs=3))
        PS = ctx.enter_context(tc.tile_pool(name='psum', bufs=1, space='PSUM'))

        mm = lambda *a, **kw: nc.tensor.matmul(*a, **kw)

        # ---- constant loads (mk first: it feeds the PE warmup) ----
        mk_s = P.tile([128, 9 * 72], AD, tag='mk')
        nc.sync.dma_start(mk_s[:], mk_d[:])
        wt_c = [P.tile([128, 3 * C], BF16, tag=f'wt{c}', name=f'wt{c}') for c in range(2)]
        for c in range(2):
            nc.scalar.dma_start(wt_c[c][:], wt_d[128 * c:128 * (c + 1), :])
        rk2_s = P.tile([2, 3 * C], BF16, tag='rk2')
        nc.scalar.dma_start(rk2_s[:], rk2_d[:])
        owt_c = [P.tile([128, C], BF16, tag=f'owt{c}', name=f'owt{c}') for c in range(2)]
        for c in range(2):
            nc.scalar.dma_start(owt_c[c][:], owt_d[128 * c:128 * (c + 1), :])
        obc_s = [P.tile([128, 1], F32, tag=f'obc{c}', name=f'obc{c}') for c in range(2)]
        for c in range(2):
            nc.scalar.dma_start(obc_s[c][:], obc_d[c])
        ones1_s = P.tile([128, 1], BF16, tag='ones1')
        nc.scalar.dma_start(ones1_s[:], ones1_d[:])
        onesr_s = P.tile([1, 128], BF16, tag='onesr')
        nc.scalar.dma_start(onesr_s[:], onesr_d[:])
        epsb_s = P.tile([3, 1], F32, tag='epsb')
        nc.scalar.dma_start(epsb_s[:], epsb_d[:])
        tm_s = P.tile([72, 8], AD, tag='tm')
        nc.scalar.dma_start(tm_s[:], tm_d[:])
        rp_s = P.tile([72, 9 * 128], AD, tag='rp')
        nc.scalar.dma_start(rp_s[:], rp_d[:])
        r72_s = P.tile([8, 72], AD, tag='r72')
        nc.scalar.dma_start(r72_s[:], r72_d[:])
        id_s = P.tile([128, 128], AD, tag='ident')
        nc.scalar.dma_start(id_s[:], id_d[:])

        # rank-2 rhs: row0 = murstd (written per LN batch), row1 = ones
        r2 = P.tile([2, NT], BF16, tag='r2')
        nc.scalar.dma_start(r2[1:2, :], onesnt_d[:])
        rstd1 = P.tile([1, NT], BF16, tag='rstd1')

        # ---- PE warmup on mk while x streams in ----
        for i in range(12):
            wps = PS.tile([128, 512], F32, tag='psO0', bufs=1, name='wps')
            mm(wps[:], mk_s[:, 0:128], mk_s[:, 0:512], start=True, stop=True)

        # ---- input x on both queues ----
        xsb = [P.tile([128, NT], F32, tag=f'xsb{c}', name=f'xsb{c}') for c in range(2)]
        for s in range(NS):
            nc.sync.dma_start(xsb[0][:, bass.ts(s, SS)], x_d[0:128, bass.ts(s, SS)])
            nc.scalar.dma_start(xsb[1][:, bass.ts(s, SS)], x_d[128:256, bass.ts(s, SS)])

        xb16 = [P.tile([128, NT], BF16, tag=f'xb16{c}', name=f'xb16{c}') for c in range(2)]
        xhat = [P.tile([128, NT], BF16, tag=f'xhat{c}', name=f'xhat{c}') for c in range(2)]
        qkv = [P.tile([128, 2, NT], AD, tag=f'qkv{t}', name=f'qkv{t}') for t in range(3)]

        # ---- per-strip: stats -> (batched LN) -> xhat -> qkv ----
        evac_rr = [0]

        def evac(dst_ap, src_ap):
            if evac_rr[0] % 2 == 0:
                nc.scalar.activation(dst_ap, src_ap, mybir.ActivationFunctionType.Copy)
            else:
                nc.vector.tensor_copy(dst_ap, src_ap)
            evac_rr[0] += 1

        NB = 3  # strips per LN batch
        mu_rows = {}
        for s in range(NS):
            sl = bass.ts(s, SS)
            b = s // NB
            if s % NB == 0:
                mu_rows[b] = (T3.tile([NB, SS], F32, tag='mu6', name='mu6', bufs=2),
                              T3.tile([NB, SS], F32, tag='m26', name='m26', bufs=2))
            mub, m2b = mu_rows[b]
            xsq = [T3.tile([128, SS], BF16, tag='xsq', name='xsq', bufs=4) for _ in range(2)]
            for c in range(2):
                nc.vector.tensor_copy(xb16[c][:, sl], xsb[c][:, sl])
                nc.scalar.square(xsq[c][:], xsb[c][:, sl])
            ps_sx = PS.tile([1, SS], F32, tag='psB', bufs=2, name='ps_sx')
            ps_sxx = PS.tile([1, SS], F32, tag='psB', bufs=2, name='ps_sxx')
            for c in range(2):
                mm(ps_sx[:], ones1_s[:], xb16[c][:, sl], start=(c == 0), stop=(c == 1))
                mm(ps_sxx[:], ones1_s[:], xsq[c][:], start=(c == 0), stop=(c == 1))
            tmu = T3.tile([1, SS], F32, tag='txe', name='tmu', bufs=4)
            tm2 = T3.tile([1, SS], F32, tag='txe', name='tm2', bufs=4)
            nc.scalar.activation(tmu[:], ps_sx[:],
                                 mybir.ActivationFunctionType.Copy, scale=1.0 / C)
            nc.vector.tensor_copy(tm2[:], ps_sxx[:])
            nc.sync.dma_start(mub[s % NB:s % NB + 1, :], tmu[:])
            nc.sync.dma_start(m2b[s % NB:s % NB + 1, :], tm2[:])

            if s % NB == NB - 1:
                # batched LN math for strips [b*NB, b*NB+NB)
                bl = bass.ts(b, NB * SS)
                musq = T3.tile([NB, SS], F32, tag='musq', bufs=2)
                nc.vector.tensor_mul(musq[:], mub[:], mub[:])
                var = T3.tile([NB, SS], F32, tag='var', bufs=2)
                # var = m2/C - musq
                nc.vector.scalar_tensor_tensor(var[:], m2b[:], 1.0 / C, musq[:],
                                               mybir.AluOpType.mult,
                                               mybir.AluOpType.subtract)
                sd = T3.tile([NB, SS], F32, tag='sd', bufs=2)
                nc.scalar.activation(sd[:], var[:], mybir.ActivationFunctionType.Sqrt,
                                     bias=epsb_s[:])
                rstd = T3.tile([NB, SS], F32, tag='rstd', bufs=2)
                nc.vector.reciprocal_approx_fast(rstd[:], sd[:])
                mrs = T3.tile([NB, SS], BF16, tag='mrs', bufs=2)
                nc.vector.tensor_mul(mrs[:], mub[:], rstd[:])
                rsb = T3.tile([NB, SS], BF16, tag='rsb', bufs=2)
                nc.vector.tensor_copy(rsb[:], rstd[:])
                nc.sync.dma_start(r2[0:1, bl], mrs[:])
                nc.sync.dma_start(rstd1[:, bl], rsb[:])
                for s2 in range(b * NB, b * NB + NB):
                    sl2 = bass.ts(s2, SS)
                    ps_rb = PS.tile([128, SS], F32, tag='psO1', bufs=1, name='ps_rb')
                    mm(ps_rb[:], onesr_s[:], rstd1[:, sl2], start=True, stop=True)
                    rstdb = T3.tile([128, SS], BF16, tag='rstdb', name='rstdb', bufs=2)
                    nc.scalar.activation(rstdb[:], ps_rb[:],
                                         mybir.ActivationFunctionType.Copy)
                    for c in range(2):
                        nc.vector.tensor_mul(xhat[c][:, sl2], xb16[c][:, sl2], rstdb[:])
                    # qkv for strip s2
                    for d in range(6):
                        dl = bass.ts(d, 128)
                        tens, half = qkv[d // 2], d % 2
                        pq = PS.tile([128, SS], F32,
                                     tag=('psA' if (s2 * 6 + d) % 2 == 0 else 'psC'),
                                     bufs=2, name='pq')
                        for c in range(2):
                            mm(pq[:], wt_c[c][:, dl], xhat[c][:, sl2],
                               start=(c == 0), stop=False)
                        mm(pq[:], rk2_s[:, dl], r2[:, sl2], start=False, stop=True)
                        evac(tens[:, half, sl2], pq[:])

        # views [128, 2, 34, 66]
        g = lambda tn: tn[:].rearrange('p h (r w) -> p h r w', w=W2)
        qg, kg, vg = g(qkv[0]), g(qkv[1]), g(qkv[2])
        xg = [xsb[c][:].rearrange('p (r w) -> p r w', w=W2) for c in range(2)]

        # ---- attention per core strip ----
        for cs in range(NCS):
            r0 = 1 + 8 * cs
            ps_sc = PS.tile([72, CS], F32, tag='psB', bufs=2, name='ps_sc')
            for t, (i, j) in enumerate(OFFS):
                prod = T3.tile([128, 2, 8, W], AD, tag='prod', bufs=4, name='prod')
                nc.vector.tensor_mul(
                    prod[:],
                    qg[:, :, r0:r0 + 8, 1:1 + W],
                    kg[:, :, r0 + i - 1:r0 + i + 7, j:j + W])
                for c in range(2):
                    mm(ps_sc[:, :], mk_s[:, bass.ts(t, 72)], prod[:, c],
                       start=(t == 0 and c == 0), stop=(t == 8 and c == 1))
            e_sb = T3.tile([72, CS], AD, tag='e_sb', bufs=2)
            nc.scalar.activation(e_sb[:], ps_sc[:], mybir.ActivationFunctionType.Exp)
            ps_den = PS.tile([8, CS], F32, tag='psA', bufs=2, name='ps_den')
            mm(ps_den[:], tm_s[:], e_sb[:], start=True, stop=True)
            rdenf = T3.tile([8, CS], F32, tag='rdenf', bufs=2)
            nc.vector.reciprocal_approx_fast(rdenf[:], ps_den[:])
            rden = T3.tile([8, CS], AD, tag='rden', bufs=2)
            nc.vector.tensor_copy(rden[:], rdenf[:])
            ps_r72 = PS.tile([72, CS], F32, tag='psA', bufs=2, name='ps_r72')
            mm(ps_r72[:], r72_s[:], rden[:], start=True, stop=True)
            rdrep = T3.tile([72, CS], AD, tag='rdrep', bufs=2)
            nc.scalar.activation(rdrep[:], ps_r72[:], mybir.ActivationFunctionType.Copy)
            attn = T3.tile([72, CS], AD, tag='attn', bufs=2)
            nc.vector.tensor_mul(attn[:], e_sb[:], rdrep[:])

            o_ps = [PS.tile([128, CS], F32, tag=f'psO{c}', bufs=1, name=f'o_ps{c}')
                    for c in range(2)]
            for t, (i, j) in enumerate(OFFS):
                erep = T3.tile([128, CS], AD, tag='erep', bufs=10)
                if erep_dma:
                    src = attn[t * 8:(t + 1) * 8, :].unsqueeze(1).broadcast_to(
                        (8, 16, CS))
                    dst = erep[:].rearrange('(h d) w -> h d w', d=16)
                    (nc.sync if t % 2 == 0 else nc.scalar).dma_start(dst, src)
                else:
                    ps_er = PS.tile([128, CS], F32, tag='psC', bufs=2, name='ps_er')
                    mm(ps_er[:], rp_s[:, bass.ts(t, 128)], attn[:],
                       start=True, stop=True)
                    nc.scalar.activation(erep[:], ps_er[:],
                                         mybir.ActivationFunctionType.Copy)
                avp = T3.tile([128, 2, 8, W], AD, tag='avp', bufs=4, name='avp')
                erv = erep[:].rearrange('p (r w) -> p r w', w=W)
                for c in range(2):
                    nc.vector.tensor_mul(
                        avp[:, c], erv,
                        vg[:, c, r0 + i - 1:r0 + i + 7, j:j + W])
                for c in range(2):
                    mm(o_ps[c][:], id_s[:], avp[:, c], start=(t == 0), stop=(t == 8))
            o_sb = [T3.tile([128, CS], BF16, tag=f'o_sb{c}', name=f'o_sb{c}', bufs=2)
                    for c in range(2)]
            for c in range(2):
                nc.scalar.activation(o_sb[c][:], o_ps[c][:],
                                     mybir.ActivationFunctionType.Copy)

            # ---- out projection; bias + residual folded into evac ----
            for db in range(2):
                op_ps = PS.tile([128, CS], F32, tag='psA', bufs=2, name='op_ps')
                for c in range(2):
                    mm(op_ps[:], owt_c[c][:, bass.ts(db, 128)], o_sb[c][:],
                       start=(c == 0), stop=(c == 1))
                ot = T3.tile([128, 8, W], F32, tag='ot', bufs=2)
                nc.vector.scalar_tensor_tensor(
                    ot[:], op_ps[:].rearrange('p (r w) -> p r w', w=W),
                    obc_s[db][:], xg[db][:, r0:r0 + 8, 1:1 + W],
                    mybir.AluOpType.add, mybir.AluOpType.add)
                (nc.sync if db == 0 else nc.scalar).dma_start(
                    out_d[bass.ts(db, 128), bass.ts(cs, CS)], ot[:])

    nc.compile()
    return nc


_NC_CACHE = {}


def _get_nc(**kw):
    key = ('nc',) + tuple(sorted(kw.items()))
    if key not in _NC_CACHE:
        _NC_CACHE[key] = build(**kw)
    return _NC_CACHE[key]


def kernel(**inputs):
    """Full-input, full-output entry point. Shards over 8 NeuronCores."""
    from concourse.bass_utils import run_bass_kernel_spmd
    nc = _get_nc()
    consts = host_prep(inputs)
    maps = core_inputs(inputs, consts)
    res = run_bass_kernel_spmd(nc, maps, core_ids=list(range(NCORE)))
    out = np.zeros((B, C, H, W), np.float32)
    for core in range(NCORE):
        b = core // 2
        r0 = (core % 2) * RPC
        out[b, :, r0:r0 + RPC, :] = res.results[core]['out'].reshape(C, RPC, W)
    return out


# revision 9
# speedup vs baseline: 1.2866x; 1.0685x over previous
"""Bass/Tile kernel for nn_CTransformer (3x3 neighborhood attention), TRN2.

Per-core layout: channel-on-partition. Core handles 32 image rows of one batch
(+1 halo row each side), width padded to 66. Tokens NT = 34*66 = 2244.

v3 design notes:
  - strip-outer pipeline: stats -> LN -> xhat -> qkv per 374-token strip, so
    DMA, stats math and qkv matmuls overlap instead of running as serial
    phases.
  - x input DMA split across both HWDGE queues (sync + scalar).
  - qkv = 3 matmul streams per (dblock, strip): 2 weight halves + one rank-2
    stream carrying the mean correction (-wsum x murstd) and the bias.
  - LN rstd / softmax denominator via reciprocal_approx_fast.
  - erep (attention-weight replication 72 -> 128 rows) via broadcast DMA on
    the HWDGE queues instead of PE matmul + psum evac.
  - q/k/v stored as [128, 2, NT]; each q*k product is one tensor_tensor op
    over both halves (FD=1024, 2x mode).
  - out-proj bias + residual folded into one scalar_tensor_tensor evac.
"""
import numpy as np
from contextlib import ExitStack

try:
    import concourse.bass as bass
except ImportError:
    import sys
    sys.path.insert(0, '/opt/trn_rl_repo')
    import concourse.bass as bass
import concourse.bacc as bacc
import concourse.tile as tile
from concourse import mybir

F32 = mybir.dt.float32
BF16 = mybir.dt.bfloat16

B, C, H, W = 4, 256, 64, 64
NHEAD, DH = 8, 32
EPS = 1e-5
NCORE = 8
RPC = 32                 # image rows per core
R2, W2 = RPC + 2, W + 2  # 34 x 66 padded grid
NT = R2 * W2             # 2244
SS = 374                 # qkv token strip (6 strips)
NS = NT // SS            # 6
CS = 512                 # core-token strip (8 image rows)
NCS = 4                  # 4 core strips
OFFS = [(i, j) for i in range(3) for j in range(3)]
AD = BF16
EREP_DMA = False          # replicate attn rows via broadcast DMA (vs PE matmul)

# repack-16 permutation: qkv row (chunk*128 + h*16 + d) <- channel (h*32 + chunk*16 + d)
PERM = np.zeros(C, dtype=np.int64)
for _k in range(2):
    for _h in range(8):
        for _d in range(16):
            PERM[_k * 128 + _h * 16 + _d] = _h * 32 + _k * 16 + _d


def host_prep(inputs):
    """Fold LN/scale/permutation into weights; build constant matrices."""
    f = np.float32
    ln_w = np.asarray(inputs['ln_w'], np.float64)
    ln_b = np.asarray(inputs['ln_b'], np.float64)
    ipw = np.asarray(inputs['in_proj_w'], np.float64)
    ipb = np.asarray(inputs['in_proj_b'], np.float64)
    opw = np.asarray(inputs['out_proj_w'], np.float64)
    opb = np.asarray(inputs['out_proj_b'], np.float64)
    scale = DH ** -0.5
    Wt = ipw * ln_w[None, :]
    bt = ipb + ipw @ ln_b
    Wt[:C] *= scale
    bt[:C] *= scale
    Wall = np.concatenate([Wt[i * C:(i + 1) * C][PERM] for i in range(3)], 0)  # [768,256]
    ball = np.concatenate([bt[i * C:(i + 1) * C][PERM] for i in range(3)], 0)
    wsum = Wall.sum(1)
    OWp = opw[:, PERM]

    import concourse.mybir as _mb
    bf = _mb.dt.np(_mb.dt.bfloat16)
    consts = {}
    consts['wt'] = np.ascontiguousarray(Wall.T).astype(bf)         # [256, 768] lhsT
    consts['rk2'] = np.stack([-wsum, ball]).astype(bf)             # [2, 768] lhsT
    consts['owt'] = np.ascontiguousarray(OWp.T).astype(bf)         # [256, 256] lhsT
    consts['obc'] = opb.astype(f).reshape(2, 128, 1)               # [2][128,1] bias cols
    consts['ones1'] = np.ones((128, 1), bf)
    consts['onesr'] = np.ones((1, 128), bf)
    consts['onesnt'] = np.ones((1, NT), bf)
    consts['epsb'] = np.full((3, 1), EPS, f)
    # scores masks: for offset t, [128, 72] with row r -> col t*8 + r//16
    mk = np.zeros((128, 9 * 72), f)
    for t in range(9):
        mk[np.arange(128), t * 72 + t * 8 + np.arange(128) // 16] = 1
    consts['mk'] = mk.astype(bf)
    # den sum: [72, 8]
    tm = np.zeros((72, 8), f)
    for t in range(9):
        for h in range(8):
            tm[t * 8 + h, h] = 1
    consts['tm'] = tm.astype(bf)
    # attn replication: [72, 9*128]
    rp = np.zeros((72, 9 * 128), f)
    for t in range(9):
        for c in range(128):
            rp[t * 8 + c // 16, t * 128 + c] = 1
    consts['rp'] = rp.astype(bf)
    # rden -> 72 rows: [8, 72]
    r72 = np.zeros((8, 72), f)
    for t in range(9):
        for h in range(8):
            r72[h, t * 8 + h] = 1
    consts['r72'] = r72.astype(bf)
    consts['ident'] = np.eye(128, dtype=f).astype(bf)
    return consts


def core_inputs(inputs, consts):
    """Build per-core in_maps (pad + slice on host)."""
    src = np.asarray(inputs['src'], np.float32)
    maps = []
    for core in range(NCORE):
        b = core // 2
        r0 = (core % 2) * RPC
        xp = np.zeros((C, R2, W2), np.float32)
        rlo, rhi = r0 - 1, r0 + RPC + 1
        slo, shi = max(rlo, 0), min(rhi, H)
        xp[:, (slo - rlo):(shi - rlo), 1:W + 1] = src[b, :, slo:shi, :]
        m = {'x': xp.reshape(C, NT)}
        m.update(consts)
        maps.append(m)
    return maps


def build(erep_dma=EREP_DMA):
    nc = bacc.Bacc('TRN2', target_bir_lowering=False, debug=False, num_devices=NCORE)

    x_d = nc.dram_tensor('x', [C, NT], F32, kind='ExternalInput')
    wt_d = nc.dram_tensor('wt', [C, 3 * C], BF16, kind='ExternalInput')
    rk2_d = nc.dram_tensor('rk2', [2, 3 * C], BF16, kind='ExternalInput')
    owt_d = nc.dram_tensor('owt', [C, C], BF16, kind='ExternalInput')
    obc_d = nc.dram_tensor('obc', [2, 128, 1], F32, kind='ExternalInput')
    ones1_d = nc.dram_tensor('ones1', [128, 1], BF16, kind='ExternalInput')
    onesr_d = nc.dram_tensor('onesr', [1, 128], BF16, kind='ExternalInput')
    onesnt_d = nc.dram_tensor('onesnt', [1, NT], BF16, kind='ExternalInput')
    epsb_d = nc.dram_tensor('epsb', [3, 1], F32, kind='ExternalInput')
    mk_d = nc.dram_tensor('mk', [128, 9 * 72], BF16, kind='ExternalInput')
    tm_d = nc.dram_tensor('tm', [72, 8], BF16, kind='ExternalInput')
    rp_d = nc.dram_tensor('rp', [72, 9 * 128], BF16, kind='ExternalInput')
    r72_d = nc.dram_tensor('r72', [8, 72], BF16, kind='ExternalInput')
    id_d = nc.dram_tensor('ident', [128, 128], BF16, kind='ExternalInput')

    out_d = nc.dram_tensor('out', [C, RPC * W], F32, kind='ExternalOutput')

    with tile.TileContext(nc) as tc, ExitStack() as ctx:
        ctx.enter_context(nc.allow_low_precision(reason='bf16 attention path'))
        P = ctx.enter_context(tc.tile_pool(name='persist', bufs=1))
        T3 = ctx.enter_context(tc.tile_pool(name='work', bufs=3))
        PS = ctx.enter_context(tc.tile_pool(name='psum', bufs=1, space='PSUM'))

        mm = lambda *a, **kw: nc.tensor.matmul(*a, **kw)

        # ---- constant loads (mk first: it feeds the PE warmup) ----
        mk_s = P.tile([128, 9 * 72], AD, tag='mk')
        nc.sync.dma_start(mk_s[:], mk_d[:])
        wt_c = [P.tile([128, 3 * C], BF16, tag=f'wt{c}', name=f'wt{c}') for c in range(2)]
        for c in range(2):
            nc.scalar.dma_start(wt_c[c][:], wt_d[128 * c:128 * (c + 1), :])
        rk2_s = P.tile([2, 3 * C], BF16, tag='rk2')
        nc.scalar.dma_start(rk2_s[:], rk2_d[:])
        owt_c = [P.tile([128, C], BF16, tag=f'owt{c}', name=f'owt{c}') for c in range(2)]
        for c in range(2):
            nc.scalar.dma_start(owt_c[c][:], owt_d[128 * c:128 * (c + 1), :])
        obc_s = [P.tile([128, 1], F32, tag=f'obc{c}', name=f'obc{c}') for c in range(2)]
        for c in range(2):
            nc.scalar.dma_start(obc_s[c][:], obc_d[c])
        ones1_s = P.tile([128, 1], BF16, tag='ones1')
        nc.scalar.dma_start(ones1_s[:], ones1_d[:])
        onesr_s = P.tile([1, 128], BF16, tag='onesr')
        nc.scalar.dma_start(onesr_s[:], onesr_d[:])
        epsb_s = P.tile([3, 1], F32, tag='epsb')
        nc.scalar.dma_start(epsb_s[:], epsb_d[:])
        tm_s = P.tile([72, 8], AD, tag='tm')
        nc.scalar.dma_start(tm_s[:], tm_d[:])
        rp_s = P.tile([72, 9 * 128], AD, tag='rp')
        nc.scalar.dma_start(rp_s[:], rp_d[:])
        r72_s = P.tile([8, 72], AD, tag='r72')
        nc.scalar.dma_start(r72_s[:], r72_d[:])
        id_s = P.tile([128, 128], AD, tag='ident')
        nc.scalar.dma_start(id_s[:], id_d[:])

        # rank-2 rhs: row0 = murstd (written per LN batch), row1 = ones
        r2 = P.tile([2, NT], BF16, tag='r2')
        nc.scalar.dma_start(r2[1:2, :], onesnt_d[:])
        rstd1 = P.tile([1, NT], BF16, tag='rstd1')

        # ---- input x on both queues ----
        xsb = [P.tile([128, NT], F32, tag=f'xsb{c}', name=f'xsb{c}') for c in range(2)]
        for s in range(NS):
            nc.sync.dma_start(xsb[0][:, bass.ts(s, SS)], x_d[0:128, bass.ts(s, SS)])
            nc.scalar.dma_start(xsb[1][:, bass.ts(s, SS)], x_d[128:256, bass.ts(s, SS)])

        xb16 = [P.tile([128, NT], BF16, tag=f'xb16{c}', name=f'xb16{c}') for c in range(2)]
        xhat = [P.tile([128, NT], BF16, tag=f'xhat{c}', name=f'xhat{c}') for c in range(2)]
        qkv = [P.tile([128, 2, NT], AD, tag=f'qkv{t}', name=f'qkv{t}') for t in range(3)]

        # ---- per-strip: stats -> (batched LN) -> xhat -> qkv ----
        evac_rr = [0]

        def evac(dst_ap, src_ap):
            if evac_rr[0] % 2 == 0:
                nc.scalar.activation(dst_ap, src_ap, mybir.ActivationFunctionType.Copy)
            else:
                nc.vector.tensor_copy(dst_ap, src_ap)
            evac_rr[0] += 1

        NB = 3  # strips per LN batch
        mu_rows = {}
        for s in range(NS):
            sl = bass.ts(s, SS)
            b = s // NB
            if s % NB == 0:
                mu_rows[b] = (T3.tile([NB, SS], F32, tag='mu6', name='mu6', bufs=2),
                              T3.tile([NB, SS], F32, tag='m26', name='m26', bufs=2))
            mub, m2b = mu_rows[b]
            xsq = [T3.tile([128, SS], BF16, tag='xsq', name='xsq', bufs=4) for _ in range(2)]
            for c in range(2):
                nc.vector.tensor_copy(xb16[c][:, sl], xsb[c][:, sl])
                nc.scalar.square(xsq[c][:], xsb[c][:, sl])
            ps_sx = PS.tile([1, SS], F32, tag='psB', bufs=2, name='ps_sx')
            ps_sxx = PS.tile([1, SS], F32, tag='psB', bufs=2, name='ps_sxx')
            for c in range(2):
                mm(ps_sx[:], ones1_s[:], xb16[c][:, sl], start=(c == 0), stop=(c == 1))
                mm(ps_sxx[:], ones1_s[:], xsq[c][:], start=(c == 0), stop=(c == 1))
            tmu = T3.tile([1, SS], F32, tag='txe', name='tmu', bufs=4)
            tm2 = T3.tile([1, SS], F32, tag='txe', name='tm2', bufs=4)
            nc.scalar.activation(tmu[:], ps_sx[:],
                                 mybir.ActivationFunctionType.Copy, scale=1.0 / C)
            nc.vector.tensor_copy(tm2[:], ps_sxx[:])
            nc.gpsimd.dma_start(mub[s % NB:s % NB + 1, :], tmu[:])
            nc.gpsimd.dma_start(m2b[s % NB:s % NB + 1, :], tm2[:])

            if s % NB == NB - 1:
                # batched LN math for strips [b*NB, b*NB+NB)
                bl = bass.ts(b, NB * SS)
                musq = T3.tile([NB, SS], F32, tag='musq', bufs=2)
                nc.vector.tensor_mul(musq[:], mub[:], mub[:])
                var = T3.tile([NB, SS], F32, tag='var', bufs=2)
                # var = m2/C - musq
                nc.vector.scalar_tensor_tensor(var[:], m2b[:], 1.0 / C, musq[:],
                                               mybir.AluOpType.mult,
                                               mybir.AluOpType.subtract)
                sd = T3.tile([NB, SS], F32, tag='sd', bufs=2)
                nc.scalar.activation(sd[:], var[:], mybir.ActivationFunctionType.Sqrt,
                                     bias=epsb_s[:])
                rstd = T3.tile([NB, SS], F32, tag='rstd', bufs=2)
                nc.vector.reciprocal_approx_fast(rstd[:], sd[:])
                mrs = T3.tile([NB, SS], BF16, tag='mrs', bufs=2)
                nc.vector.tensor_mul(mrs[:], mub[:], rstd[:])
                rsb = T3.tile([NB, SS], BF16, tag='rsb', bufs=2)
                nc.vector.tensor_copy(rsb[:], rstd[:])
                nc.gpsimd.dma_start(r2[0:1, bl], mrs[:])
                nc.gpsimd.dma_start(rstd1[:, bl], rsb[:])
                for s2 in range(b * NB, b * NB + NB):
                    sl2 = bass.ts(s2, SS)
                    ps_rb = PS.tile([128, SS], F32, tag='psO1', bufs=1, name='ps_rb')
                    mm(ps_rb[:], onesr_s[:], rstd1[:, sl2], start=True, stop=True)
                    rstdb = T3.tile([128, SS], BF16, tag='rstdb', name='rstdb', bufs=2)
                    nc.scalar.activation(rstdb[:], ps_rb[:],
                                         mybir.ActivationFunctionType.Copy)
                    for c in range(2):
                        nc.vector.tensor_mul(xhat[c][:, sl2], xb16[c][:, sl2], rstdb[:])
                    # qkv for strip s2
                    for d in range(6):
                        dl = bass.ts(d, 128)
                        tens, half = qkv[d // 2], d % 2
                        pq = PS.tile([128, SS], F32,
                                     tag=('psA' if (s2 * 6 + d) % 2 == 0 else 'psC'),
                                     bufs=2, name='pq')
                        for c in range(2):
                            mm(pq[:], wt_c[c][:, dl], xhat[c][:, sl2],
                               start=(c == 0), stop=False)
                        mm(pq[:], rk2_s[:, dl], r2[:, sl2], start=False, stop=True)
                        evac(tens[:, half, sl2], pq[:])

        # views [128, 2, 34, 66]
        g = lambda tn: tn[:].rearrange('p h (r w) -> p h r w', w=W2)
        qg, kg, vg = g(qkv[0]), g(qkv[1]), g(qkv[2])
        xg = [xsb[c][:].rearrange('p (r w) -> p r w', w=W2) for c in range(2)]

        # ---- attention per core strip ----
        for cs in range(NCS):
            r0 = 1 + 8 * cs
            ps_sc = PS.tile([72, CS], F32, tag='psB', bufs=2, name='ps_sc')
            for t, (i, j) in enumerate(OFFS):
                prod = T3.tile([128, 2, 8, W], AD, tag='prod', bufs=4, name='prod')
                nc.vector.tensor_mul(
                    prod[:],
                    qg[:, :, r0:r0 + 8, 1:1 + W],
                    kg[:, :, r0 + i - 1:r0 + i + 7, j:j + W])
                for c in range(2):
                    mm(ps_sc[:, :], mk_s[:, bass.ts(t, 72)], prod[:, c],
                       start=(t == 0 and c == 0), stop=(t == 8 and c == 1))
            e_sb = T3.tile([72, CS], AD, tag='e_sb', bufs=2)
            nc.scalar.activation(e_sb[:], ps_sc[:], mybir.ActivationFunctionType.Exp)
            ps_den = PS.tile([8, CS], F32, tag='psA', bufs=2, name='ps_den')
            mm(ps_den[:], tm_s[:], e_sb[:], start=True, stop=True)
            rdenf = T3.tile([8, CS], F32, tag='rdenf', bufs=2)
            nc.vector.reciprocal_approx_fast(rdenf[:], ps_den[:])
            rden = T3.tile([8, CS], AD, tag='rden', bufs=2)
            nc.vector.tensor_copy(rden[:], rdenf[:])
            ps_r72 = PS.tile([72, CS], F32, tag='psA', bufs=2, name='ps_r72')
            mm(ps_r72[:], r72_s[:], rden[:], start=True, stop=True)
            rdrep = T3.tile([72, CS], AD, tag='rdrep', bufs=2)
            nc.scalar.activation(rdrep[:], ps_r72[:], mybir.ActivationFunctionType.Copy)
            attn = T3.tile([72, CS], AD, tag='attn', bufs=2)
            nc.vector.tensor_mul(attn[:], e_sb[:], rdrep[:])

            o_ps = [PS.tile([128, CS], F32, tag=f'psO{c}', bufs=1, name=f'o_ps{c}')
                    for c in range(2)]
            for t, (i, j) in enumerate(OFFS):
                erep = T3.tile([128, CS], AD, tag='erep', bufs=10)
                if erep_dma:
                    src = attn[t * 8:(t + 1) * 8, :].unsqueeze(1).broadcast_to(
                        (8, 16, CS))
                    dst = erep[:].rearrange('(h d) w -> h d w', d=16)
                    (nc.sync if t % 2 == 0 else nc.scalar).dma_start(dst, src)
                else:
                    ps_er = PS.tile([128, CS], F32, tag='psC', bufs=2, name='ps_er')
                    mm(ps_er[:], rp_s[:, bass.ts(t, 128)], attn[:],
                       start=True, stop=True)
                    nc.scalar.activation(erep[:], ps_er[:],
                                         mybir.ActivationFunctionType.Copy)
                avp = T3.tile([128, 2, 8, W], AD, tag='avp', bufs=4, name='avp')
                erv = erep[:].rearrange('p (r w) -> p r w', w=W)
                for c in range(2):
                    nc.vector.tensor_mul(
                        avp[:, c], erv,
                        vg[:, c, r0 + i - 1:r0 + i + 7, j:j + W])
                for c in range(2):
                    mm(o_ps[c][:], id_s[:], avp[:, c], start=(t == 0), stop=(t == 8))
            o_sb = [T3.tile([128, CS], BF16, tag=f'o_sb{c}', name=f'o_sb{c}', bufs=2)
                    for c in range(2)]
            for c in range(2):
                nc.scalar.activation(o_sb[c][:], o_ps[c][:],
                                     mybir.ActivationFunctionType.Copy)

            # ---- out projection; bias + residual folded into evac ----
            for db in range(2):
                op_ps = PS.tile([128, CS], F32, tag='psA', bufs=2, name='op_ps')
                for c in range(2):
                    mm(op_ps[:], owt_c[c][:, bass.ts(db, 128)], o_sb[c][:],
                       start=(c == 0), stop=(c == 1))
                ot = T3.tile([128, 8, W], F32, tag='ot', bufs=2)
                nc.vector.scalar_tensor_tensor(
                    ot[:], op_ps[:].rearrange('p (r w) -> p r w', w=W),
                    obc_s[db][:], xg[db][:, r0:r0 + 8, 1:1 + W],
                    mybir.AluOpType.add, mybir.AluOpType.add)
                nc.sync.dma_start(
                    out_d[bass.ts(db, 128), bass.ts(cs, CS)], ot[:])

    nc.compile()
    return nc


_NC_CACHE = {}


def _get_nc(**kw):
    key = ('nc',) + tuple(sorted(kw.items()))
    if key not in _NC_CACHE:
        _NC_CACHE[key] = build(**kw)
    return _NC_CACHE[key]


def kernel(**inputs):
    """Full-input, full-output entry point. Shards over 8 NeuronCores."""
    from concourse.bass_utils import run_bass_kernel_spmd
    nc = _get_nc()
    consts = host_prep(inputs)
    maps = core_inputs(inputs, consts)
    res = run_bass_kernel_spmd(nc, maps, core_ids=list(range(NCORE)))
    out = np.zeros((B, C, H, W), np.float32)
    for core in range(NCORE):
        b = core // 2
        r0 = (core % 2) * RPC
        out[b, :, r0:r0 + RPC, :] = res.results[core]['out'].reshape(C, RPC, W)
    return out


# revision 11
# speedup vs baseline: 1.3086x; 1.0171x over previous
"""Bass/Tile kernel for nn_CTransformer (3x3 neighborhood attention), TRN2.

Per-core layout: channel-on-partition. Core handles 32 image rows of one batch
(+1 halo row each side), width padded to 66. Tokens NT = 34*66 = 2244.

v3 design notes:
  - strip-outer pipeline: stats -> LN -> xhat -> qkv per 374-token strip, so
    DMA, stats math and qkv matmuls overlap instead of running as serial
    phases.
  - x input DMA split across both HWDGE queues (sync + scalar).
  - qkv = 3 matmul streams per (dblock, strip): 2 weight halves + one rank-2
    stream carrying the mean correction (-wsum x murstd) and the bias.
  - LN rstd / softmax denominator via reciprocal_approx_fast.
  - erep (attention-weight replication 72 -> 128 rows) via broadcast DMA on
    the HWDGE queues instead of PE matmul + psum evac.
  - q/k/v stored as [128, 2, NT]; each q*k product is one tensor_tensor op
    over both halves (FD=1024, 2x mode).
  - out-proj bias + residual folded into one scalar_tensor_tensor evac.
"""
import numpy as np
from contextlib import ExitStack

try:
    import concourse.bass as bass
except ImportError:
    import sys
    sys.path.insert(0, '/opt/trn_rl_repo')
    import concourse.bass as bass
import concourse.bacc as bacc
import concourse.tile as tile
from concourse import mybir

F32 = mybir.dt.float32
BF16 = mybir.dt.bfloat16

B, C, H, W = 4, 256, 64, 64
NHEAD, DH = 8, 32
EPS = 1e-5
NCORE = 8
RPC = 32                 # image rows per core
R2, W2 = RPC + 2, W + 2  # 34 x 66 padded grid
NT = R2 * W2             # 2244
SS = 374                 # qkv token strip (6 strips)
NS = NT // SS            # 6
CS = 512                 # core-token strip (8 image rows)
NCS = 4                  # 4 core strips
OFFS = [(i, j) for i in range(3) for j in range(3)]
AD = BF16
EREP_DMA = False          # replicate attn rows via broadcast DMA (vs PE matmul)

# repack-16 permutation: qkv row (chunk*128 + h*16 + d) <- channel (h*32 + chunk*16 + d)
PERM = np.zeros(C, dtype=np.int64)
for _k in range(2):
    for _h in range(8):
        for _d in range(16):
            PERM[_k * 128 + _h * 16 + _d] = _h * 32 + _k * 16 + _d


def host_prep(inputs):
    """Fold LN/scale/permutation into weights; build constant matrices."""
    f = np.float32
    ln_w = np.asarray(inputs['ln_w'], np.float64)
    ln_b = np.asarray(inputs['ln_b'], np.float64)
    ipw = np.asarray(inputs['in_proj_w'], np.float64)
    ipb = np.asarray(inputs['in_proj_b'], np.float64)
    opw = np.asarray(inputs['out_proj_w'], np.float64)
    opb = np.asarray(inputs['out_proj_b'], np.float64)
    scale = DH ** -0.5
    Wt = ipw * ln_w[None, :]
    bt = ipb + ipw @ ln_b
    Wt[:C] *= scale
    bt[:C] *= scale
    Wall = np.concatenate([Wt[i * C:(i + 1) * C][PERM] for i in range(3)], 0)  # [768,256]
    ball = np.concatenate([bt[i * C:(i + 1) * C][PERM] for i in range(3)], 0)
    wsum = Wall.sum(1)
    OWp = opw[:, PERM]

    import concourse.mybir as _mb
    bf = _mb.dt.np(_mb.dt.bfloat16)
    consts = {}
    consts['wt'] = np.ascontiguousarray(Wall.T).astype(bf)         # [256, 768] lhsT
    consts['rk2'] = np.stack([-wsum, ball]).astype(bf)             # [2, 768] lhsT
    consts['owt'] = np.ascontiguousarray(OWp.T).astype(bf)         # [256, 256] lhsT
    consts['obc'] = opb.astype(f).reshape(2, 128, 1)               # [2][128,1] bias cols
    consts['ones1'] = np.ones((128, 1), bf)
    consts['onesr'] = np.ones((1, 128), bf)
    consts['onesnt'] = np.ones((1, NT), bf)
    consts['epsb'] = np.full((3, 1), EPS, f)
    # scores masks: for offset t, [128, 72] with row r -> col t*8 + r//16
    mk = np.zeros((128, 9 * 72), f)
    for t in range(9):
        mk[np.arange(128), t * 72 + t * 8 + np.arange(128) // 16] = 1
    consts['mk'] = mk.astype(bf)
    # den sum: [72, 8]
    tm = np.zeros((72, 8), f)
    for t in range(9):
        for h in range(8):
            tm[t * 8 + h, h] = 1
    consts['tm'] = tm.astype(bf)
    # attn replication: [72, 9*128]
    rp = np.zeros((72, 9 * 128), f)
    for t in range(9):
        for c in range(128):
            rp[t * 8 + c // 16, t * 128 + c] = 1
    consts['rp'] = rp.astype(bf)
    # rden -> 128 rows: [8, 128] (head h -> rows 16h..16h+15)
    r128 = np.zeros((8, 128), f)
    for h in range(8):
        r128[h, 16 * h:16 * (h + 1)] = 1
    consts['r128'] = r128.astype(bf)
    consts['ident'] = np.eye(128, dtype=f).astype(bf)
    return consts


def core_inputs(inputs, consts):
    """Build per-core in_maps (pad + slice on host)."""
    src = np.asarray(inputs['src'], np.float32)
    maps = []
    for core in range(NCORE):
        b = core // 2
        r0 = (core % 2) * RPC
        xp = np.zeros((C, R2, W2), np.float32)
        rlo, rhi = r0 - 1, r0 + RPC + 1
        slo, shi = max(rlo, 0), min(rhi, H)
        xp[:, (slo - rlo):(shi - rlo), 1:W + 1] = src[b, :, slo:shi, :]
        m = {'x': xp.reshape(C, NT)}
        m.update(consts)
        maps.append(m)
    return maps


def build(erep_dma=EREP_DMA):
    nc = bacc.Bacc('TRN2', target_bir_lowering=False, debug=False, num_devices=NCORE)

    x_d = nc.dram_tensor('x', [C, NT], F32, kind='ExternalInput')
    wt_d = nc.dram_tensor('wt', [C, 3 * C], BF16, kind='ExternalInput')
    rk2_d = nc.dram_tensor('rk2', [2, 3 * C], BF16, kind='ExternalInput')
    owt_d = nc.dram_tensor('owt', [C, C], BF16, kind='ExternalInput')
    obc_d = nc.dram_tensor('obc', [2, 128, 1], F32, kind='ExternalInput')
    ones1_d = nc.dram_tensor('ones1', [128, 1], BF16, kind='ExternalInput')
    onesr_d = nc.dram_tensor('onesr', [1, 128], BF16, kind='ExternalInput')
    onesnt_d = nc.dram_tensor('onesnt', [1, NT], BF16, kind='ExternalInput')
    epsb_d = nc.dram_tensor('epsb', [3, 1], F32, kind='ExternalInput')
    mk_d = nc.dram_tensor('mk', [128, 9 * 72], BF16, kind='ExternalInput')
    tm_d = nc.dram_tensor('tm', [72, 8], BF16, kind='ExternalInput')
    rp_d = nc.dram_tensor('rp', [72, 9 * 128], BF16, kind='ExternalInput')
    r128_d = nc.dram_tensor('r128', [8, 128], BF16, kind='ExternalInput')
    id_d = nc.dram_tensor('ident', [128, 128], BF16, kind='ExternalInput')

    out_d = nc.dram_tensor('out', [C, RPC * W], F32, kind='ExternalOutput')

    with tile.TileContext(nc) as tc, ExitStack() as ctx:
        ctx.enter_context(nc.allow_low_precision(reason='bf16 attention path'))
        P = ctx.enter_context(tc.tile_pool(name='persist', bufs=1))
        T3 = ctx.enter_context(tc.tile_pool(name='work', bufs=3))
        PS = ctx.enter_context(tc.tile_pool(name='psum', bufs=1, space='PSUM'))

        mm = lambda *a, **kw: nc.tensor.matmul(*a, **kw)

        # ---- input x first (both queues), then constants ----
        xsb = [P.tile([128, NT], F32, tag=f'xsb{c}', name=f'xsb{c}') for c in range(2)]
        ones1_s = P.tile([128, 1], BF16, tag='ones1')
        epsb_s = P.tile([3, 1], F32, tag='epsb')
        for s in range(NS):
            nc.sync.dma_start(xsb[0][:, bass.ts(s, SS)], x_d[0:128, bass.ts(s, SS)])
            nc.scalar.dma_start(xsb[1][:, bass.ts(s, SS)], x_d[128:256, bass.ts(s, SS)])
            if s == 0:
                nc.sync.dma_start(ones1_s[:], ones1_d[:])
                nc.scalar.dma_start(epsb_s[:], epsb_d[:])
        wt_c = [P.tile([128, 3 * C], BF16, tag=f'wt{c}', name=f'wt{c}') for c in range(2)]
        nc.sync.dma_start(wt_c[0][:], wt_d[0:128, :])
        nc.scalar.dma_start(wt_c[1][:], wt_d[128:256, :])
        rk2_s = P.tile([2, 3 * C], BF16, tag='rk2')
        nc.sync.dma_start(rk2_s[:], rk2_d[:])
        onesr_s = P.tile([1, 128], BF16, tag='onesr')
        nc.scalar.dma_start(onesr_s[:], onesr_d[:])
        owt_c = [P.tile([128, C], BF16, tag=f'owt{c}', name=f'owt{c}') for c in range(2)]
        nc.sync.dma_start(owt_c[0][:], owt_d[0:128, :])
        nc.scalar.dma_start(owt_c[1][:], owt_d[128:256, :])
        obc_s = [P.tile([128, 1], F32, tag=f'obc{c}', name=f'obc{c}') for c in range(2)]
        nc.sync.dma_start(obc_s[0][:], obc_d[0])
        nc.scalar.dma_start(obc_s[1][:], obc_d[1])
        mk_s = P.tile([128, 9 * 72], AD, tag='mk')
        nc.sync.dma_start(mk_s[:], mk_d[:])
        tm_s = P.tile([72, 8], AD, tag='tm')
        nc.scalar.dma_start(tm_s[:], tm_d[:])
        rp_s = P.tile([72, 9 * 128], AD, tag='rp')
        nc.sync.dma_start(rp_s[:], rp_d[:])
        r128_s = P.tile([8, 128], AD, tag='r128')
        nc.scalar.dma_start(r128_s[:], r128_d[:])
        id_s = P.tile([128, 128], AD, tag='ident')
        nc.sync.dma_start(id_s[:], id_d[:])
        r2 = P.tile([2, NT], BF16, tag='r2')
        nc.scalar.dma_start(r2[1:2, :], onesnt_d[:])
        rstd1 = P.tile([1, NT], BF16, tag='rstd1')

        xb16 = [P.tile([128, NT], BF16, tag=f'xb16{c}', name=f'xb16{c}') for c in range(2)]
        xhat = [P.tile([128, NT], BF16, tag=f'xhat{c}', name=f'xhat{c}') for c in range(2)]
        qkv = [P.tile([128, 2, NT], AD, tag=f'qkv{t}', name=f'qkv{t}') for t in range(3)]

        # ---- per-strip: stats -> (batched LN) -> xhat -> qkv ----
        evac_rr = [0]

        def evac(dst_ap, src_ap):
            if evac_rr[0] % 2 == 0:
                nc.scalar.activation(dst_ap, src_ap, mybir.ActivationFunctionType.Copy)
            else:
                nc.vector.tensor_copy(dst_ap, src_ap)
            evac_rr[0] += 1

        NB = 3  # strips per LN batch
        mu_rows = {}
        for s in range(NS):
            sl = bass.ts(s, SS)
            b = s // NB
            if s % NB == 0:
                mu_rows[b] = (T3.tile([NB, SS], F32, tag='mu6', name='mu6', bufs=2),
                              T3.tile([NB, SS], F32, tag='m26', name='m26', bufs=2))
            mub, m2b = mu_rows[b]
            xsq = [T3.tile([128, SS], BF16, tag='xsq', name='xsq', bufs=4) for _ in range(2)]
            for c in range(2):
                nc.vector.tensor_copy(xb16[c][:, sl], xsb[c][:, sl])
                nc.scalar.square(xsq[c][:], xsb[c][:, sl])
            ps_sx = PS.tile([1, SS], F32, tag='psB', bufs=2, name='ps_sx')
            ps_sxx = PS.tile([1, SS], F32, tag='psB', bufs=2, name='ps_sxx')
            for c in range(2):
                mm(ps_sx[:], ones1_s[:], xb16[c][:, sl], start=(c == 0), stop=(c == 1))
                mm(ps_sxx[:], ones1_s[:], xsq[c][:], start=(c == 0), stop=(c == 1))
            tmu = T3.tile([1, SS], F32, tag='txe', name='tmu', bufs=4)
            tm2 = T3.tile([1, SS], F32, tag='txe', name='tm2', bufs=4)
            nc.scalar.activation(tmu[:], ps_sx[:],
                                 mybir.ActivationFunctionType.Copy, scale=1.0 / C)
            nc.vector.tensor_copy(tm2[:], ps_sxx[:])
            nc.sync.dma_start(mub[s % NB:s % NB + 1, :], tmu[:])
            nc.scalar.dma_start(m2b[s % NB:s % NB + 1, :], tm2[:])

            if s % NB == NB - 1:
                # batched LN math for strips [b*NB, b*NB+NB)
                bl = bass.ts(b, NB * SS)
                musq = T3.tile([NB, SS], F32, tag='musq', bufs=2)
                nc.vector.tensor_mul(musq[:], mub[:], mub[:])
                var = T3.tile([NB, SS], F32, tag='var', bufs=2)
                # var = m2/C - musq
                nc.vector.scalar_tensor_tensor(var[:], m2b[:], 1.0 / C, musq[:],
                                               mybir.AluOpType.mult,
                                               mybir.AluOpType.subtract)
                sd = T3.tile([NB, SS], F32, tag='sd', bufs=2)
                nc.scalar.activation(sd[:], var[:], mybir.ActivationFunctionType.Sqrt,
                                     bias=epsb_s[:])
                rstd = T3.tile([NB, SS], F32, tag='rstd', bufs=2)
                nc.vector.reciprocal_approx_fast(rstd[:], sd[:])
                mrs = T3.tile([NB, SS], BF16, tag='mrs', bufs=2)
                nc.vector.tensor_mul(mrs[:], mub[:], rstd[:])
                rsb = T3.tile([NB, SS], BF16, tag='rsb', bufs=2)
                nc.vector.tensor_copy(rsb[:], rstd[:])
                nc.sync.dma_start(r2[0:1, bl], mrs[:])
                nc.scalar.dma_start(rstd1[:, bl], rsb[:])
                strips = list(range(b * NB, b * NB + NB))
                for s2 in strips:
                    sl2 = bass.ts(s2, SS)
                    ps_rb = PS.tile([128, SS], F32, tag='psO1', bufs=1, name='ps_rb')
                    mm(ps_rb[:], onesr_s[:], rstd1[:, sl2], start=True, stop=True)
                    rstdb = T3.tile([128, SS], BF16, tag='rstdb', name='rstdb', bufs=2)
                    nc.scalar.activation(rstdb[:], ps_rb[:],
                                         mybir.ActivationFunctionType.Copy)
                    for c in range(2):
                        nc.vector.tensor_mul(xhat[c][:, sl2], xb16[c][:, sl2], rstdb[:])
                # qkv: per dblock, run each lhsT across all 3 strips back-to-back
                for d in range(6):
                    dl = bass.ts(d, 128)
                    tens, half = qkv[d // 2], d % 2
                    pqs = {}
                    for k, s2 in enumerate(strips):
                        pqs[s2] = PS.tile([128, SS], F32,
                                          tag=('psA' if k % 2 == 0 else 'psC'),
                                          bufs=2, name='pq')
                    for c in range(2):
                        for s2 in strips:
                            mm(pqs[s2][:], wt_c[c][:, dl], xhat[c][:, bass.ts(s2, SS)],
                               start=(c == 0), stop=False)
                    for s2 in strips:
                        mm(pqs[s2][:], rk2_s[:, dl], r2[:, bass.ts(s2, SS)],
                           start=False, stop=True)
                    for s2 in strips:
                        evac(tens[:, half, bass.ts(s2, SS)], pqs[s2][:])

        # views [128, 2, 34, 66]
        g = lambda tn: tn[:].rearrange('p h (r w) -> p h r w', w=W2)
        qg, kg, vg = g(qkv[0]), g(qkv[1]), g(qkv[2])
        xg = [xsb[c][:].rearrange('p (r w) -> p r w', w=W2) for c in range(2)]

        # ---- attention per core strip ----
        for cs in range(NCS):
            r0 = 1 + 8 * cs
            ps_sc = PS.tile([72, CS], F32, tag='psB', bufs=2, name='ps_sc')
            for t, (i, j) in enumerate(OFFS):
                prod = T3.tile([128, 2, 8, W], AD, tag='prod', bufs=4, name='prod')
                nc.vector.tensor_mul(
                    prod[:],
                    qg[:, :, r0:r0 + 8, 1:1 + W],
                    kg[:, :, r0 + i - 1:r0 + i + 7, j:j + W])
                for c in range(2):
                    mm(ps_sc[:, :], mk_s[:, bass.ts(t, 72)], prod[:, c],
                       start=(t == 0 and c == 0), stop=(t == 8 and c == 1))
            e_sb = T3.tile([72, CS], AD, tag='e_sb', bufs=2)
            nc.scalar.activation(e_sb[:], ps_sc[:], mybir.ActivationFunctionType.Exp)
            # denominator path runs concurrent with the erep/avp rounds below;
            # normalization is applied at o evac time
            ps_den = PS.tile([8, CS], F32, tag='psA', bufs=2, name='ps_den')
            mm(ps_den[:], tm_s[:], e_sb[:], start=True, stop=True)
            rdenf = T3.tile([8, CS], F32, tag='rdenf', bufs=2)
            nc.vector.reciprocal_approx_fast(rdenf[:], ps_den[:])
            rden = T3.tile([8, CS], AD, tag='rden', bufs=2)
            nc.vector.tensor_copy(rden[:], rdenf[:])
            ps_r72 = PS.tile([128, CS], F32, tag='psA', bufs=2, name='ps_r72')
            mm(ps_r72[:], r128_s[:], rden[:], start=True, stop=True)
            rdrep = T3.tile([128, CS], AD, tag='rdrep', bufs=2)
            nc.scalar.activation(rdrep[:], ps_r72[:], mybir.ActivationFunctionType.Copy)
            attn = e_sb

            o_ps = [PS.tile([128, CS], F32, tag=f'psO{c}', bufs=1, name=f'o_ps{c}')
                    for c in range(2)]
            for t, (i, j) in enumerate(OFFS):
                erep = T3.tile([128, CS], AD, tag='erep', bufs=10)
                if erep_dma:
                    src = attn[t * 8:(t + 1) * 8, :].unsqueeze(1).broadcast_to(
                        (8, 16, CS))
                    dst = erep[:].rearrange('(h d) w -> h d w', d=16)
                    (nc.sync if t % 2 == 0 else nc.scalar).dma_start(dst, src)
                else:
                    ps_er = PS.tile([128, CS], F32, tag='psC', bufs=2, name='ps_er')
                    mm(ps_er[:], rp_s[:, bass.ts(t, 128)], attn[:],
                       start=True, stop=True)
                    nc.scalar.activation(erep[:], ps_er[:],
                                         mybir.ActivationFunctionType.Copy)
                avp = T3.tile([128, 2, 8, W], AD, tag='avp', bufs=4, name='avp')
                erv = erep[:].rearrange('p (r w) -> p r w', w=W)
                for c in range(2):
                    nc.vector.tensor_mul(
                        avp[:, c], erv,
                        vg[:, c, r0 + i - 1:r0 + i + 7, j:j + W])
                for c in range(2):
                    mm(o_ps[c][:], id_s[:], avp[:, c], start=(t == 0), stop=(t == 8))
            o_sb = [T3.tile([128, CS], BF16, tag=f'o_sb{c}', name=f'o_sb{c}', bufs=2)
                    for c in range(2)]
            for c in range(2):
                nc.vector.tensor_mul(o_sb[c][:], o_ps[c][:], rdrep[:])

            # ---- out projection; bias + residual folded into evac ----
            for db in range(2):
                op_ps = PS.tile([128, CS], F32, tag='psA', bufs=2, name='op_ps')
                for c in range(2):
                    mm(op_ps[:], owt_c[c][:, bass.ts(db, 128)], o_sb[c][:],
                       start=(c == 0), stop=(c == 1))
                ot = T3.tile([128, 8, W], F32, tag='ot', bufs=2)
                nc.vector.scalar_tensor_tensor(
                    ot[:], op_ps[:].rearrange('p (r w) -> p r w', w=W),
                    obc_s[db][:], xg[db][:, r0:r0 + 8, 1:1 + W],
                    mybir.AluOpType.add, mybir.AluOpType.add)
                nc.sync.dma_start(
                    out_d[bass.ts(db, 128), bass.ts(cs, CS)], ot[:])

    nc.compile()
    return nc


_NC_CACHE = {}


def _get_nc(**kw):
    key = ('nc',) + tuple(sorted(kw.items()))
    if key not in _NC_CACHE:
        _NC_CACHE[key] = build(**kw)
    return _NC_CACHE[key]


def kernel(**inputs):
    """Full-input, full-output entry point. Shards over 8 NeuronCores."""
    from concourse.bass_utils import run_bass_kernel_spmd
    nc = _get_nc()
    consts = host_prep(inputs)
    maps = core_inputs(inputs, consts)
    res = run_bass_kernel_spmd(nc, maps, core_ids=list(range(NCORE)))
    out = np.zeros((B, C, H, W), np.float32)
    for core in range(NCORE):
        b = core // 2
        r0 = (core % 2) * RPC
        out[b, :, r0:r0 + RPC, :] = res.results[core]['out'].reshape(C, RPC, W)
    return out
